# revision 1
# baseline (speedup 1.0000x reference)
"""DeltaNet block kernel for Trainium2, data-parallel over batch (8 cores).

Strategy (per core, one batch element; L=1024, H=1024, E=2048):
  - LN affine params, the pos_embed rank-1 term, and the beta concat trick are
    folded into effective weights on the host (exact algebra, no approximation).
  - The delta-rule pair of einsums is rewritten in "attention form":
        out = (q @ k^T) @ (beta*v)     [saves 2x FLOPs since L < E]
  - All matmuls run in bf16 with fp32 PSUM accumulation.
  - Depthwise conv1d(k=3) runs on the tensor engine as 3 accumulating
    diagonal matmuls over shifted access patterns (channels on partitions).
  - Layout changes use the DMA xbar transpose engine (bf16).
"""

import os
import sys

import numpy as np

sys.path.insert(0, "/opt/trn_rl_repo")

import ml_dtypes  # noqa: E402

import concourse.bass as bass  # noqa: E402
import concourse.mybir as mybir  # noqa: E402
import concourse.tile as tile  # noqa: E402
from concourse.bass_utils import run_bass_kernel_spmd  # noqa: E402

BF16 = mybir.dt.bfloat16
F32 = mybir.dt.float32
AF = mybir.ActivationFunctionType
ALU = mybir.AluOpType

B, L, H, E = 8, 1024, 1024, 2048
P = 128
LC = L // P   # 8  l-chunks
KC = H // P   # 8  h-chunks
EC = E // P   # 16 e-chunks
JC = 4 * H // P  # 32 intermediate chunks
NQ = 512      # matmul / psum free dim
EPS = 1e-5

# test.py can flip these before calling kernel()
TRACE = False
LAST = {}


def _build_program(attn_scale: float, stop_after: str | None = None, repeat: int = 1):
    nc = bass.Bass("TRN2", target_bir_lowering=False)

    x_d = nc.dram_tensor("x", [P, LC, H], F32, kind="ExternalInput")
    wqkq_d = nc.dram_tensor("wqkq", [P, KC, E], BF16, kind="ExternalInput")
    wqkk_d = nc.dram_tensor("wqkk", [P, KC, E], BF16, kind="ExternalInput")
    wv_d = nc.dram_tensor("wv", [P, KC, E], BF16, kind="ExternalInput")
    wb_d = nc.dram_tensor("wb", [P, KC, E], BF16, kind="ExternalInput")
    wout_d = nc.dram_tensor("wout", [P, EC, H], BF16, kind="ExternalInput")
    w1a_d = nc.dram_tensor("w1a", [P, KC, E], BF16, kind="ExternalInput")
    w1b_d = nc.dram_tensor("w1b", [P, KC, E], BF16, kind="ExternalInput")
    w2a_d = nc.dram_tensor("w2a", [P, JC, NQ], BF16, kind="ExternalInput")
    w2b_d = nc.dram_tensor("w2b", [P, JC, NQ], BF16, kind="ExternalInput")
    cdiag_d = nc.dram_tensor("cdiag", [P, EC, 3, P], BF16, kind="ExternalInput")
    bv_d = nc.dram_tensor("bv", [P, EC], F32, kind="ExternalInput")
    bb_d = nc.dram_tensor("bb", [P, EC], F32, kind="ExternalInput")
    b1_d = nc.dram_tensor("b1c", [P, JC], F32, kind="ExternalInput")
    y_d = nc.dram_tensor("y", [P, LC, H], F32, kind="ExternalOutput")
    xnew_d = nc.dram_tensor("xnew_scratch", [P, LC, H], F32)

    with tile.TileContext(nc) as tc:
        with (
            tc.tile_pool(name="consts", bufs=1) as consts,
            tc.tile_pool(name="wt", bufs=2) as wtp,
            tc.tile_pool(name="bigA", bufs=2) as bigA,
            tc.tile_pool(name="bigB", bufs=2) as bigB,
            tc.tile_pool(name="qkc", bufs=4) as qkc,
            tc.tile_pool(name="vbc", bufs=3) as vbc,
            tc.tile_pool(name="xyc", bufs=3) as xyc,
            tc.tile_pool(name="st", bufs=4) as stp,
            tc.tile_pool(name="psum", bufs=6, space="PSUM") as psum,
        ):
            zero_t = consts.tile([P, 1], F32)
            nc.vector.memset(zero_t, 0.0)
            nc.const_aps.aps[(F32, 0.0)] = zero_t[:]
            eps_t = consts.tile([P, 1], F32)
            nc.vector.memset(eps_t, EPS)

            cdiag = consts.tile([P, EC, 3, P], BF16)
            nc.sync.dma_start(cdiag, cdiag_d[:])
            bv_sb = consts.tile([P, EC], F32)
            nc.sync.dma_start(bv_sb, bv_d[:])
            bb_sb = consts.tile([P, EC], F32)
            nc.sync.dma_start(bb_sb, bb_d[:])
            b1_sb = consts.tile([P, JC], F32)
            nc.sync.dma_start(b1_sb, b1_d[:])

            def ln_stats(src, n):
                """src: [P, n] -> (mean, rstd) [P,1] f32 each."""
                nsub = n // 512
                stt = stp.tile([P, nsub, 6], F32, tag="bnst")
                src3 = src.rearrange("p (s f) -> p s f", s=nsub)
                for s in range(nsub):
                    nc.vector.bn_stats(stt[:, s, :], src3[:, s, :])
                mv = stp.tile([P, 2], F32, tag="mv")
                nc.vector.bn_aggr(mv, stt)
                rstd = stp.tile([P, 1], F32, tag="rstd")
                nc.scalar.activation(rstd, mv[:, 1:2], AF.Sqrt, bias=eps_t[:])
                nc.vector.reciprocal(rstd, rstd)
                return mv[:, 0:1], rstd

            def standardize(dst, src, n):
                mean, rstd = ln_stats(src, n)
                nc.vector.tensor_scalar(
                    dst, src, mean, rstd, op0=ALU.subtract, op1=ALU.mult
                )

            def conv3(ps, row, hf, dg):
                """3-tap depthwise conv into psum ps [P,NQ].  row [P, L] is one
                e-chunk, l on the free dim, no padding.  Tap order: full-width
                center tap opens the accumulation group (start=True clears the
                bank), the edge-truncated shifted tap runs in the middle, and a
                full-width shifted tap closes the group."""
                base = hf * NQ
                nc.tensor.matmul(
                    ps, dg[:, 1, :], row[:, base : base + NQ],
                    start=True, stop=False,
                )
                if hf == 0:
                    nc.tensor.matmul(
                        ps[:, 1:NQ], dg[:, 0, :], row[:, 0 : NQ - 1],
                        start=False, stop=False, skip_group_check=True,
                    )
                    nc.tensor.matmul(
                        ps, dg[:, 2, :], row[:, 1 : NQ + 1],
                        start=False, stop=True, skip_group_check=True,
                    )
                else:
                    nc.tensor.matmul(
                        ps[:, 0 : NQ - 1], dg[:, 2, :], row[:, base + 1 : L],
                        start=False, stop=False, skip_group_check=True,
                    )
                    nc.tensor.matmul(
                        ps, dg[:, 0, :], row[:, base - 1 : base - 1 + NQ],
                        start=False, stop=True, skip_group_check=True,
                    )

            def dump3(src_ap):
                """Debug: cast+copy a [P, 8, 1024]-shaped AP into y and stop."""
                for c in range(src_ap.shape[1]):
                    tmp = xyc.tile([P, H], F32, tag="xyc")
                    nc.vector.tensor_copy(tmp, src_ap[:, c, :])
                    nc.sync.dma_start(y_d[:, c, :], tmp)

            for _rep in range(repeat):
                # ---------------- P0: LN1(x) -> hT [P, KC, L] bf16 ----------------
                hT = bigB.tile([P, KC, L], BF16, tag="bigB")
                for lc in range(LC):
                    xt = xyc.tile([P, H], F32, tag="xyc")
                    nc.sync.dma_start(xt, x_d[:, lc, :])
                    z = vbc.tile([P, H], BF16, tag="vbc")
                    standardize(z, xt, H)
                    nc.sync.dma_start_transpose(hT[:, :, lc * P : (lc + 1) * P], z)

                if stop_after == "h":
                    dump3(hT)
            
                # ---------------- P3: q,k + silu + normalize-mix -> qT,kT --------
                qT = bigA.tile([P, EC, L], BF16, tag="bigA")
                kT = bigA.tile([P, EC, L], BF16, tag="bigA")
                wq = wtp.tile([P, KC, E], BF16, tag="wt")
                nc.sync.dma_start(wq, wqkq_d[:])
                wk = wtp.tile([P, KC, E], BF16, tag="wt")
                nc.sync.dma_start(wk, wqkk_d[:])
                for lc in range(LC):
                    qs = qkc.tile([P, E], BF16, tag="qkc")
                    ks = qkc.tile([P, E], BF16, tag="qkc")
                    for wu, dst in ((wq, qs), (wk, ks)):
                        for n in range(E // NQ):
                            ps = psum.tile([P, NQ], F32, tag="ps")
                            for kc in range(KC):
                                nc.tensor.matmul(
                                    ps,
                                    hT[:, kc, lc * P : (lc + 1) * P],
                                    wu[:, kc, n * NQ : (n + 1) * NQ],
                                    start=(kc == 0),
                                    stop=(kc == KC - 1),
                                )
                            nc.scalar.activation(dst[:, n * NQ : (n + 1) * NQ], ps, AF.Silu)
                    ssq_q = stp.tile([P, 1], F32, tag="ssq")
                    ssq_k = stp.tile([P, 1], F32, tag="ssq")
                    q1 = qkc.tile([P, E], BF16, tag="qkc")
                    k1 = qkc.tile([P, E], BF16, tag="qkc")
                    # q1/k1 double as dead-store scratch for the Square pass
                    nc.scalar.activation(q1, qs, AF.Square, accum_out=ssq_q)
                    nc.scalar.activation(k1, ks, AF.Square, accum_out=ssq_k)
                    for ssq in (ssq_q, ssq_k):
                        nc.scalar.activation(ssq, ssq, AF.Sqrt)
                        nc.vector.tensor_scalar_max(ssq, ssq, 1e-12)
                        nc.vector.reciprocal(ssq, ssq)
                    nc.vector.tensor_scalar_mul(q1, qs, ssq_q)   # q_hat
                    nc.vector.tensor_scalar_mul(k1, ks, ssq_k)   # k_hat
                    nc.vector.tensor_scalar_mul(ks, ks, 0.1)     # 0.1*k_silu (in place)
                    nc.vector.tensor_add(q1, q1, ks)             # q1 = q_hat + 0.1 k_s
                    nc.sync.dma_start_transpose(qT[:, :, lc * P : (lc + 1) * P], q1)
                    nc.vector.tensor_scalar_mul(q1, q1, 0.1)     # after transpose read
                    nc.vector.tensor_add(k1, k1, q1)             # k1 = k_hat + 0.1 q1
                    nc.sync.dma_start_transpose(kT[:, :, lc * P : (lc + 1) * P], k1)

                if stop_after == "qT":
                    dump3(qT[:, 0:8, :])
                    return nc
                # ---------------- P4: conv q,k in place (diag matmuls) -----------
                for tz in (qT, kT):
                    for ec in range(EC):
                        ps0 = psum.tile([P, NQ], F32, tag="ps")
                        conv3(ps0, tz[:, ec, :], 0, cdiag[:, ec])
                        ps1 = psum.tile([P, NQ], F32, tag="ps")
                        conv3(ps1, tz[:, ec, :], 1, cdiag[:, ec])
                        # in-place evac; Tile orders these after both halves' reads
                        nc.scalar.copy(tz[:, ec, 0:NQ], ps0)
                        nc.scalar.copy(tz[:, ec, NQ : 2 * NQ], ps1)

                if stop_after == "qTc":
                    dump3(qT[:, 0:8, :])
                    return nc
                # ---------------- P5: A^T = (k_c)^T-weighted q matmul ------------
                AT = bigB.tile([P, LC, L], BF16, tag="bigB")
                for lpc in range(LC):
                    for hf in range(2):
                        ps = psum.tile([P, NQ], F32, tag="ps")
                        for ec in range(EC):
                            nc.tensor.matmul(
                                ps,
                                kT[:, ec, lpc * P : (lpc + 1) * P],
                                qT[:, ec, hf * NQ : (hf + 1) * NQ],
                                start=(ec == 0),
                                stop=(ec == EC - 1),
                            )
                        if attn_scale == 1.0:
                            nc.scalar.copy(AT[:, lpc, hf * NQ : (hf + 1) * NQ], ps)
                        else:
                            nc.scalar.activation(
                                AT[:, lpc, hf * NQ : (hf + 1) * NQ], ps, AF.Copy,
                                scale=float(attn_scale),
                            )

                if stop_after == "AT":
                    dump3(AT)
                    return nc
                # ---------------- P1v: v,beta + gelu/sigmoid + conv + transpose --
                wv = wtp.tile([P, KC, E], BF16, tag="wt")
                nc.sync.dma_start(wv, wv_d[:])
                wb = wtp.tile([P, KC, E], BF16, tag="wt")
                nc.sync.dma_start(wb, wb_d[:])
                v_new = bigA.tile([P, LC, E], BF16, tag="bigA")
                for ec in range(EC):
                    vt = vbc.tile([P, L], BF16, tag="vbc")
                    bt = vbc.tile([P, L], BF16, tag="vbc")
                    for hf in range(2):
                        ps = psum.tile([P, NQ], F32, tag="ps")
                        for kc in range(KC):
                            nc.tensor.matmul(
                                ps,
                                wv[:, kc, ec * P : (ec + 1) * P],
                                hT[:, kc, hf * NQ : (hf + 1) * NQ],
                                start=(kc == 0),
                                stop=(kc == KC - 1),
                            )
                        nc.scalar.activation(
                            vt[:, hf * NQ : (hf + 1) * NQ], ps, AF.Gelu,
                            bias=bv_sb[:, ec : ec + 1],
                        )
                        ps2 = psum.tile([P, NQ], F32, tag="ps")
                        for kc in range(KC):
                            nc.tensor.matmul(
                                ps2,
                                wb[:, kc, ec * P : (ec + 1) * P],
                                hT[:, kc, hf * NQ : (hf + 1) * NQ],
                                start=(kc == 0),
                                stop=(kc == KC - 1),
                            )
                        nc.scalar.activation(
                            bt[:, hf * NQ : (hf + 1) * NQ], ps2, AF.Sigmoid,
                            bias=bb_sb[:, ec : ec + 1],
                        )
                    nc.vector.tensor_scalar(bt, bt, 0.9, 0.1, op0=ALU.mult, op1=ALU.add)
                    vnt = vbc.tile([P, L], BF16, tag="vbc")
                    for hf in range(2):
                        ps = psum.tile([P, NQ], F32, tag="ps")
                        conv3(ps, vt, hf, cdiag[:, ec])
                        nc.vector.tensor_mul(
                            vnt[:, hf * NQ : (hf + 1) * NQ], ps,
                            bt[:, hf * NQ : (hf + 1) * NQ],
                        )
                    nc.sync.dma_start_transpose(v_new[:, :, ec * P : (ec + 1) * P], vnt)

                if stop_after == "v_new":
                    dump3(v_new[:, :, 0:1024])
                    return nc
                # ---------------- P6: out = A @ v_new  -> attn [P, LC, E] --------
                attn = bigA.tile([P, LC, E], BF16, tag="bigA")
                for lc in range(LC):
                    for f in range(E // NQ):
                        ps = psum.tile([P, NQ], F32, tag="ps")
                        for lpc in range(LC):
                            nc.tensor.matmul(
                                ps,
                                AT[:, lpc, lc * P : (lc + 1) * P],
                                v_new[:, lpc, f * NQ : (f + 1) * NQ],
                                start=(lpc == 0),
                                stop=(lpc == LC - 1),
                            )
                        nc.scalar.copy(attn[:, lc, f * NQ : (f + 1) * NQ], ps)

                if stop_after == "attn":
                    dump3(attn[:, :, 0:1024])
                    return nc
                # ---------------- P7: LN2 -> z2 (in place) -> z2T ----------------
                z2T = bigA.tile([P, EC, L], BF16, tag="bigA")
                for lc in range(LC):
                    standardize(attn[:, lc, :], attn[:, lc, :], E)
                    nc.sync.dma_start_transpose(
                        z2T[:, :, lc * P : (lc + 1) * P], attn[:, lc, :]
                    )

                if stop_after == "z2T":
                    dump3(z2T[:, 0:8, :])
                    return nc
                # ---------------- P8: proj_out + residual -> xnew (DRAM) ---------
                wo = wtp.tile([P, EC, H], BF16, tag="wt")
                nc.sync.dma_start(wo, wout_d[:])
                for lc in range(LC):
                    xt = xyc.tile([P, H], F32, tag="xyc")
                    nc.sync.dma_start(xt, x_d[:, lc, :])
                    xn = xyc.tile([P, H], F32, tag="xyc")
                    for hc in range(H // NQ):
                        ps = psum.tile([P, NQ], F32, tag="ps")
                        for ec in range(EC):
                            nc.tensor.matmul(
                                ps,
                                z2T[:, ec, lc * P : (lc + 1) * P],
                                wo[:, ec, hc * NQ : (hc + 1) * NQ],
                                start=(ec == 0),
                                stop=(ec == EC - 1),
                            )
                        nc.vector.tensor_add(
                            xn[:, hc * NQ : (hc + 1) * NQ], ps,
                            xt[:, hc * NQ : (hc + 1) * NQ],
                        )
                    nc.sync.dma_start(xnew_d[:, lc, :], xn)

                if stop_after == "xnew":
                    for lc2 in range(LC):
                        nc.sync.dma_start(y_d[:, lc2, :], xnew_d[:, lc2, :])
                    return nc
                # ---------------- P9: h2 = LN1(xnew) -> h2T ----------------------
                h2T = bigB.tile([P, KC, L], BF16, tag="bigB")
                for lc in range(LC):
                    xt = xyc.tile([P, H], F32, tag="xyc")
                    nc.sync.dma_start(xt, xnew_d[:, lc, :])
                    z = vbc.tile([P, H], BF16, tag="vbc")
                    standardize(z, xt, H)
                    nc.sync.dma_start_transpose(h2T[:, :, lc * P : (lc + 1) * P], z)

                if stop_after == "h2T":
                    dump3(h2T)
                    return nc
                # ---------------- P10: mlp1 (gelu) -> ugT ------------------------
                ug_a = bigA.tile([P, JC // 2, L], BF16, tag="bigA")
                ug_b = bigA.tile([P, JC // 2, L], BF16, tag="bigA")
                w1a = wtp.tile([P, KC, E], BF16, tag="wt")
                nc.sync.dma_start(w1a, w1a_d[:])
                w1b = wtp.tile([P, KC, E], BF16, tag="wt")
                nc.sync.dma_start(w1b, w1b_d[:])
                for half, (w1u, ugx) in enumerate(((w1a, ug_a), (w1b, ug_b))):
                    for jx in range(JC // 2):
                        jc = half * (JC // 2) + jx
                        for hf in range(2):
                            ps = psum.tile([P, NQ], F32, tag="ps")
                            for kc in range(KC):
                                nc.tensor.matmul(
                                    ps,
                                    w1u[:, kc, jx * P : (jx + 1) * P],
                                    h2T[:, kc, hf * NQ : (hf + 1) * NQ],
                                    start=(kc == 0),
                                    stop=(kc == KC - 1),
                                )
                            nc.scalar.activation(
                                ugx[:, jx, hf * NQ : (hf + 1) * NQ], ps, AF.Gelu,
                                bias=b1_sb[:, jc : jc + 1],
                            )

                if stop_after == "ugT":
                    dump3(ug_a[:, 0:8, :])
                    return nc
                # ---------------- P11: mlp2 + residual -> y ----------------------
                w2a = wtp.tile([P, JC, NQ], BF16, tag="wt")
                nc.sync.dma_start(w2a, w2a_d[:])
                w2b = wtp.tile([P, JC, NQ], BF16, tag="wt")
                nc.sync.dma_start(w2b, w2b_d[:])
                for lc in range(LC):
                    xt = xyc.tile([P, H], F32, tag="xyc")
                    nc.sync.dma_start(xt, xnew_d[:, lc, :])
                    yt = xyc.tile([P, H], F32, tag="xyc")
                    for hc, w2u in enumerate((w2a, w2b)):
                        ps = psum.tile([P, NQ], F32, tag="ps")
                        for jc in range(JC):
                            ugx = ug_a if jc < JC // 2 else ug_b
                            nc.tensor.matmul(
                                ps,
                                ugx[:, jc % (JC // 2), lc * P : (lc + 1) * P],
                                w2u[:, jc, :],
                                start=(jc == 0),
                                stop=(jc == JC - 1),
                            )
                        nc.vector.tensor_add(
                            yt[:, hc * NQ : (hc + 1) * NQ], ps,
                            xt[:, hc * NQ : (hc + 1) * NQ],
                        )
                    nc.sync.dma_start(y_d[:, lc, :], yt)
    return nc


def _legalize_waits(nc, limit=1):
    """This walrus build rejects instructions carrying more than a couple of
    sync waits ("Too many sync wait commands").  Split excess waits onto
    same-engine NOPs inserted immediately before the instruction — engine
    program order makes this equivalent."""
    cnt = 0
    for fn in nc.m.functions:
        for bb in fn.blocks:
            insts = bb.instructions
            fixes = []  # (index, [nops])
            for idx, ins in enumerate(insts):
                si = ins.sync_info
                if si is None or not si.on_wait or len(si.on_wait) <= limit:
                    continue
                waits = list(si.on_wait)
                excess, keep = waits[:-limit], waits[-limit:]
                nops = []
                for j in range(0, len(excess), limit):
                    nop = mybir.InstNoOp(name=f"WFIX-{cnt}", text_hint="waitfix")
                    cnt += 1
                    nop.engine = ins.engine
                    nop.sync_info = mybir.SyncInfo(
                        on_wait=excess[j : j + limit], on_update=[]
                    )
                    nops.append(nop)
                si.on_wait = keep
                fixes.append((idx, nops))
            for idx, nops in reversed(fixes):
                for nop in reversed(nops):
                    insts.insert(idx, nop)
    return cnt


def _to_pchunk(a2d, nchunk):
    """[R, C] with R = nchunk*128 -> [128, nchunk, C] (p-major layout)."""
    R, C = a2d.shape
    return np.ascontiguousarray(
        a2d.reshape(nchunk, P, C).transpose(1, 0, 2)
    )


def _prep_inputs(inputs):
    f32 = lambda a: np.asarray(a, np.float32)
    bf = lambda a: np.ascontiguousarray(a.astype(ml_dtypes.bfloat16))

    x = f32(inputs["x"])
    ln1_w, ln1_b = f32(inputs["ln1_w"]), f32(inputs["ln1_b"])
    ln2_w, ln2_b = f32(inputs["ln2_w"]), f32(inputs["ln2_b"])
    w_qkv, b_qkv = f32(inputs["w_qkv"]), f32(inputs["b_qkv"])
    w_out, b_out = f32(inputs["w_out"]), f32(inputs["b_out"])
    rel_pos = f32(inputs["rel_pos"])
    w_beta, b_beta = f32(inputs["w_beta"]), f32(inputs["b_beta"])
    w1, b1 = f32(inputs["w1"]), f32(inputs["b1"])
    w2, b2 = f32(inputs["w2"]), f32(inputs["b2"])
    conv_w = f32(inputs["conv_w"])
    attn_scale = float(np.asarray(inputs["attn_scale"]).reshape(-1)[0])

    # biases we cannot fold for free must be zero (true for this problem's
    # setup_inputs); the general path would add broadcast-row adds.
    assert not np.any(b_qkv[: 2 * E]), "nonzero q/k bias not supported"
    assert not np.any(b_out) and not np.any(b2), "nonzero row bias not supported"

    # fold LN affine into the consuming matmuls: y = z @ (W*g)^T + (b + W@c)
    wqkv_e = w_qkv * ln1_w[None, :]
    bqkv_e = b_qkv + w_qkv @ ln1_b
    wq_e, wk_e, wv_e = wqkv_e[:E], wqkv_e[E : 2 * E], wqkv_e[2 * E :]
    bv_e = bqkv_e[2 * E :]

    # beta: comb=[h, pos_info] trick -> rank-1 update, then LN fold
    p_bar = rel_pos[:L].mean(0)
    s = w_beta[:, H:].sum(1)
    wb_raw = w_beta[:, :H] + np.outer(s, p_bar)
    wb_e = wb_raw * ln1_w[None, :]
    bb_e = b_beta + wb_raw @ ln1_b

    wout_e = w_out * ln2_w[None, :]
    # b_out + w_out @ ln2_b must be zero for the no-row-bias fast path
    bout_e = b_out + w_out @ ln2_b
    assert np.allclose(bout_e, 0.0), "nonzero folded out bias not supported"

    w1_e = w1 * ln1_w[None, :]
    b1_e = b1 + w1 @ ln1_b

    # conv diag blocks: cd[p, ec, t, m] = conv_w[ec*128+p, 0, t] if p==m else 0
    cd = np.zeros((P, EC, 3, P), np.float32)
    idx = np.arange(P)
    cd[idx, :, :, idx] = conv_w[:, 0, :].reshape(EC, P, 3).transpose(1, 0, 2)

    wqk = np.concatenate([wq_e, wk_e], axis=0)  # [2E, H]
    wqkT = _to_pchunk(wqk.T, KC)                # [128, KC, 2E]

    shared = {
        "wqkq": bf(wqkT[:, :, :E]),
        "wqkk": bf(wqkT[:, :, E:]),
        "wv": bf(_to_pchunk(wv_e.T, KC)),
        "wb": bf(_to_pchunk(wb_e.T, KC)),
        "wout": bf(_to_pchunk(wout_e.T, EC)),
        "w1a": bf(_to_pchunk(w1_e.T, KC)[:, :, :E]),
        "w1b": bf(_to_pchunk(w1_e.T, KC)[:, :, E:]),
        "w2a": bf(_to_pchunk(w2.T, JC)[:, :, :NQ]),
        "w2b": bf(_to_pchunk(w2.T, JC)[:, :, NQ:]),
        "cdiag": bf(cd),
        "bv": np.ascontiguousarray(bv_e.reshape(EC, P).T),
        "bb": np.ascontiguousarray(bb_e.reshape(EC, P).T),
        "b1c": np.ascontiguousarray(b1_e.reshape(JC, P).T),
    }
    in_maps = []
    for b in range(B):
        m = dict(shared)
        m["x"] = np.ascontiguousarray(
            x[b].reshape(LC, P, H).transpose(1, 0, 2)
        )
        in_maps.append(m)
    return in_maps, attn_scale


def kernel(**inputs) -> np.ndarray:
    in_maps, attn_scale = _prep_inputs(inputs)
    nc = _build_program(attn_scale)
    _legalize_waits(nc)
    res = run_bass_kernel_spmd(
        nc, in_maps, core_ids=list(range(B)), trace=TRACE
    )
    LAST["exec_time_ns"] = res.exec_time_ns
    LAST["results"] = res
    out = np.empty((B, L, H), np.float32)
    for b in range(B):
        yb = np.asarray(res.results[b]["y"])  # [128, LC, H]
        out[b] = yb.transpose(1, 0, 2).reshape(L, H)
    return out



# revision 7
# speedup vs baseline: 1.2225x; 1.2225x over previous
"""DeltaNet block kernel for Trainium2, data-parallel over batch (8 cores).

v2 strategy (per core, one batch element; L=1024, H=1024, E=2048):
  - LN affine params and the pos_embed rank-1 term are folded into effective
    weights on the host (exact algebra).  All row biases are exactly zero for
    this problem (asserted) so no bias plumbing.
  - Delta-rule einsums in attention form: out = (q @ k^T) @ (beta*v).
  - All matmuls bf16 with fp32 PSUM accumulation.
  - Activation-table discipline: the scalar engine only ever uses
    {Sqrt} -> {Silu} -> {Sigmoid, Erf} -> {Copy, Sqrt} -> {Gelu}, one table
    load per phase instead of one per alternation (gelu for v is computed via
    erf, which lives in the sigmoid table set; the 0.5 factor is folded into
    the beta gate: beta' = 0.45*sigmoid + 0.05).
  - q/k row-norm rsqrt via bit-trick + 2 Newton steps on the vector engine
    (avoids Sqrt-table thrash inside the silu phase).
  - Depthwise conv(k=3): q/k on the tensor engine (3 accumulating diagonal
    matmuls over shifted APs); v on the vector engine (3 fused
    scalar_tensor_tensor taps over a zero-guarded row).
  - LN2 / pre-MLP LN1 statistics are accumulated during the psum evacuations
    (activation accum_out + a squared-sum scalar_tensor_tensor), so the LN
    phases collapse into the producing phases.
  - x + attn_out residual (xnew) stays in SBUF in bf16; no DRAM round trip.
  - Weight DMAs are column-chunked and double-rotate through two 32KB SBUF
    slots (s1: wq->wv->wout->w1b->w2b, s2: wk->wb->w1a->w2a); each load is
    WAR-gated on the previous tenant's last read, so every transfer hides
    under compute.
"""

import sys

import numpy as np

sys.path.insert(0, "/opt/trn_rl_repo")

import ml_dtypes  # noqa: E402

import concourse.bass as bass  # noqa: E402
import concourse.mybir as mybir  # noqa: E402
import concourse.tile as tile  # noqa: E402
from concourse.bass_utils import run_bass_kernel_spmd  # noqa: E402

BF16 = mybir.dt.bfloat16
F32 = mybir.dt.float32
I32 = mybir.dt.int32
AF = mybir.ActivationFunctionType
ALU = mybir.AluOpType
AX = mybir.AxisListType

B, L, H, E = 8, 1024, 1024, 2048
P = 128
LC = L // P    # 8
KC = H // P    # 8
EC = E // P    # 16
JC = 4 * H // P  # 32
NQ = 512
EPS = 1e-5
RSQRT_MAGIC = 0x5F3759DF
INV_SQRT2 = 0.7071067811865476

TRACE = False
LAST = {}


def _build_program(attn_scale: float, stop_after: str | None = None):
    nc = bass.Bass("TRN2", target_bir_lowering=False)

    x_d = nc.dram_tensor("x", [P, LC, H], F32, kind="ExternalInput")
    wq_d = nc.dram_tensor("wq", [P, 4, KC, NQ], BF16, kind="ExternalInput")
    wk_d = nc.dram_tensor("wk", [P, 4, KC, NQ], BF16, kind="ExternalInput")
    wv_d = nc.dram_tensor("wv", [P, 4, KC, NQ], BF16, kind="ExternalInput")
    wb_d = nc.dram_tensor("wb", [P, 4, KC, NQ], BF16, kind="ExternalInput")
    wo_d = nc.dram_tensor("wo", [P, 2, EC, NQ], BF16, kind="ExternalInput")
    w1a_d = nc.dram_tensor("w1a", [P, 4, KC, NQ], BF16, kind="ExternalInput")
    w1b_d = nc.dram_tensor("w1b", [P, 4, KC, NQ], BF16, kind="ExternalInput")
    w2a_d = nc.dram_tensor("w2a", [P, 2, 16, NQ], BF16, kind="ExternalInput")
    w2b_d = nc.dram_tensor("w2b", [P, 2, 16, NQ], BF16, kind="ExternalInput")
    cdiag_d = nc.dram_tensor("cdiag", [P, EC, 3, P], BF16, kind="ExternalInput")
    cwv_d = nc.dram_tensor("cwv", [P, 3, EC], F32, kind="ExternalInput")
    y_d = nc.dram_tensor("y", [P, LC, H], F32, kind="ExternalOutput")

    with tile.TileContext(nc) as tc:
        with (
            tc.tile_pool(name="consts", bufs=1) as consts,
            tc.tile_pool(name="wts", bufs=1) as wts,
            tc.tile_pool(name="acts", bufs=1) as acts,
            tc.tile_pool(name="work", bufs=2) as work,
            tc.tile_pool(name="psum", bufs=8, space="PSUM") as psum,
        ):
            zero_t = consts.tile([P, 1], F32)
            nc.vector.memset(zero_t, 0.0)
            nc.const_aps.aps[(F32, 0.0)] = zero_t[:]
            eps_t = consts.tile([P, 1], F32)
            nc.vector.memset(eps_t, EPS)
            c15_t = consts.tile([P, 1], F32)
            nc.vector.memset(c15_t, 1.5)
            cwv = consts.tile([P, 3, EC], F32)
            nc.sync.dma_start(cwv, cwv_d[:])
            dead1 = consts.tile([P, 1], BF16)

            def rsqrt_vec(ssq):
                """[P,1] f32 ssq -> [P,1] f32 rsqrt via bit trick + 2 Newton."""
                se = work.tile([P, 1], F32, tag="nt", bufs=16)
                nc.vector.tensor_scalar_add(se, ssq, 1e-20)
                hh = work.tile([P, 1], F32, tag="nt", bufs=16)
                nc.vector.tensor_scalar_mul(hh, se, -0.5)
                r = work.tile([P, 1], F32, tag="nt", bufs=16)
                nc.vector.tensor_scalar(
                    r.bitcast(I32), se.bitcast(I32), 1, -1,
                    op0=ALU.arith_shift_right, op1=ALU.bitwise_xor,
                )
                nc.vector.tensor_scalar_add(
                    r.bitcast(I32), r.bitcast(I32), RSQRT_MAGIC + 1
                )
                for _ in range(2):
                    yy = work.tile([P, 1], F32, tag="nt", bufs=16)
                    nc.vector.tensor_mul(yy, r, r)
                    ww = work.tile([P, 1], F32, tag="nt", bufs=16)
                    nc.vector.scalar_tensor_tensor(
                        ww, yy, hh, c15_t, op0=ALU.mult, op1=ALU.add
                    )
                    r2 = work.tile([P, 1], F32, tag="nt", bufs=16)
                    nc.vector.tensor_mul(r2, ww, r)
                    r = r2
                return r

            def rstd_from_sums(ssum, sqsum, n, scl=1.0):
                """[P,1] sums of x and x^2 -> (mean, rstd) [P,1] f32.
                scl: values were scaled by scl in ssum but raw in sqsum."""
                mean = work.tile([P, 1], F32, tag="st1", bufs=8)
                nc.vector.tensor_scalar_mul(mean, ssum, 1.0 / n)
                ex2 = work.tile([P, 1], F32, tag="st1", bufs=8)
                nc.vector.tensor_scalar_mul(ex2, sqsum, scl * scl / n)
                var = work.tile([P, 1], F32, tag="st1", bufs=8)
                nc.vector.tensor_mul(var, mean, mean)
                nc.vector.tensor_sub(var, ex2, var)
                r = work.tile([P, 1], F32, tag="st1", bufs=8)
                nc.scalar.activation(r, var, AF.Sqrt, bias=eps_t[:])
                nc.vector.reciprocal(r, r)
                return mean, r

            def conv3(ps, row, hf, dg):
                """3-tap PE conv into psum ps [P,NQ]; row [P,L] one e-chunk."""
                base = hf * NQ
                nc.tensor.matmul(
                    ps, dg[:, 1, :], row[:, base : base + NQ],
                    start=True, stop=False,
                )
                if hf == 0:
                    nc.tensor.matmul(
                        ps[:, 1:NQ], dg[:, 0, :], row[:, 0 : NQ - 1],
                        start=False, stop=False, skip_group_check=True,
                    )
                    nc.tensor.matmul(
                        ps, dg[:, 2, :], row[:, 1 : NQ + 1],
                        start=False, stop=True, skip_group_check=True,
                    )
                else:
                    nc.tensor.matmul(
                        ps[:, 0 : NQ - 1], dg[:, 2, :], row[:, base + 1 : L],
                        start=False, stop=False, skip_group_check=True,
                    )
                    nc.tensor.matmul(
                        ps, dg[:, 0, :], row[:, base - 1 : base - 1 + NQ],
                        start=False, stop=True, skip_group_check=True,
                    )

            def dump3(src_ap):
                for c in range(src_ap.shape[1]):
                    tmp = work.tile([P, H], F32, tag="xy")
                    nc.vector.tensor_copy(tmp, src_ap[:, c, :])
                    nc.sync.dma_start(y_d[:, c, :], tmp)

            # ---------------- P0: LN1(x) -> hT [P, KC, L] bf16 ----------------
            hT = acts.tile([P, KC, L], BF16, tag="hT")
            for lc in range(LC):
                xt = work.tile([P, H], F32, tag="xy")
                nc.sync.dma_start(xt, x_d[:, lc, :])
                bnst = work.tile([P, 2, 6], F32, tag="bnst")
                x3 = xt.rearrange("p (s f) -> p s f", s=2)
                for s in range(2):
                    nc.vector.bn_stats(bnst[:, s, :], x3[:, s, :])
                mv = work.tile([P, 2], F32, tag="mv")
                nc.vector.bn_aggr(mv, bnst)
                rstd = work.tile([P, 1], F32, tag="st1", bufs=8)
                nc.scalar.activation(rstd, mv[:, 1:2], AF.Sqrt, bias=eps_t[:])
                nc.vector.reciprocal(rstd, rstd)
                z = work.tile([P, H], BF16, tag="h2c")
                nc.vector.tensor_scalar(
                    z, xt, mv[:, 0:1], rstd, op0=ALU.subtract, op1=ALU.mult
                )
                nc.sync.dma_start_transpose(hT[:, :, lc * P : (lc + 1) * P], z)

            if stop_after == "h":
                dump3(hT)
                return nc

            # ---------------- P3: q,k + silu + normalize-mix -> qT,kT --------
            qT = acts.tile([P, EC, L], BF16, tag="big1")
            kT = acts.tile([P, EC, L], BF16, tag="big2")
            wq_t = wts.tile([P, 4, KC, NQ], BF16, tag="s1")
            wk_t = wts.tile([P, 4, KC, NQ], BF16, tag="s2")
            for n in range(4):
                nc.scalar.dma_start(wq_t[:, n], wq_d[:, n])
            for n in range(4):
                nc.scalar.dma_start(wk_t[:, n], wk_d[:, n])
            for lc in range(LC):
                qs = work.tile([P, E], BF16, tag="qk", bufs=3)
                ks = work.tile([P, E], BF16, tag="qk", bufs=3)
                for wu, dst in ((wq_t, qs), (wk_t, ks)):
                    for n in range(4):
                        ps = psum.tile([P, NQ], F32, tag="ps")
                        for kc in range(KC):
                            nc.tensor.matmul(
                                ps,
                                hT[:, kc, lc * P : (lc + 1) * P],
                                wu[:, n, kc, :],
                                start=(kc == 0),
                                stop=(kc == KC - 1),
                            )
                        nc.scalar.activation(dst[:, n * NQ : (n + 1) * NQ], ps, AF.Silu)
                s = work.tile([P, E], BF16, tag="qk", bufs=3)
                ssq_q = work.tile([P, 1], F32, tag="st1", bufs=8)
                nc.vector.scalar_tensor_tensor(
                    s, qs, 1.0, qs, op0=ALU.bypass, op1=ALU.mult,
                    accum_out=ssq_q,
                )
                ssq_k = work.tile([P, 1], F32, tag="st1", bufs=8)
                nc.vector.scalar_tensor_tensor(
                    s, ks, 1.0, ks, op0=ALU.bypass, op1=ALU.mult,
                    accum_out=ssq_k,
                )
                rq = rsqrt_vec(ssq_q)
                rk = rsqrt_vec(ssq_k)
                nc.vector.tensor_scalar_mul(s, ks, 0.1)
                nc.vector.scalar_tensor_tensor(
                    qs, qs, rq, s, op0=ALU.mult, op1=ALU.add
                )
                nc.sync.dma_start_transpose(qT[:, :, lc * P : (lc + 1) * P], qs)
                nc.vector.tensor_scalar_mul(s, qs, 0.1)
                nc.vector.scalar_tensor_tensor(
                    ks, ks, rk, s, op0=ALU.mult, op1=ALU.add
                )
                nc.sync.dma_start_transpose(kT[:, :, lc * P : (lc + 1) * P], ks)

            if stop_after == "qT":
                dump3(qT[:, 0:8, :])
                return nc

            # ---------------- P4: conv q,k in place (diag matmuls) -----------
            for ec in range(EC):
                cdg = work.tile([P, 3, P], BF16, tag="cdg", bufs=2)
                nc.scalar.dma_start(cdg, cdiag_d[:, ec])
                for tz in (qT, kT):
                    ps0 = psum.tile([P, NQ], F32, tag="ps")
                    conv3(ps0, tz[:, ec, :], 0, cdg)
                    ps1 = psum.tile([P, NQ], F32, tag="ps")
                    conv3(ps1, tz[:, ec, :], 1, cdg)
                    nc.scalar.copy(tz[:, ec, 0:NQ], ps0)
                    nc.scalar.copy(tz[:, ec, NQ : 2 * NQ], ps1)

            if stop_after == "qTc":
                dump3(qT[:, 0:8, :])
                return nc

            # ---------------- P5: AT = (q @ k^T)^T chunks --------------------
            AT = acts.tile([P, LC, L], BF16, tag="big3")
            for lpc in range(LC):
                for hf in range(2):
                    ps = psum.tile([P, NQ], F32, tag="ps")
                    for ec in range(EC):
                        nc.tensor.matmul(
                            ps,
                            kT[:, ec, lpc * P : (lpc + 1) * P],
                            qT[:, ec, hf * NQ : (hf + 1) * NQ],
                            start=(ec == 0),
                            stop=(ec == EC - 1),
                        )
                    nc.scalar.copy(AT[:, lpc, hf * NQ : (hf + 1) * NQ], ps)

            if stop_after == "AT":
                dump3(AT)
                return nc

            # ---------------- P1v: v,beta + erf-gelu + vec conv + transpose --
            wv_t = wts.tile([P, 4, KC, NQ], BF16, tag="s1")
            wb_t = wts.tile([P, 4, KC, NQ], BF16, tag="s2")
            for n in range(4):
                nc.scalar.dma_start(wv_t[:, n], wv_d[:, n])
            for n in range(4):
                nc.scalar.dma_start(wb_t[:, n], wb_d[:, n])
            v_new = acts.tile([P, LC, E], BF16, tag="big2")
            for ec in range(EC):
                vt = work.tile([P, L + 2], BF16, tag="vt")
                nc.vector.memset(vt[:, 0:1], 0.0)
                nc.vector.memset(vt[:, L + 1 : L + 2], 0.0)
                for hf in range(2):
                    ps = psum.tile([P, NQ], F32, tag="ps")
                    for kc in range(KC):
                        nc.tensor.matmul(
                            ps,
                            wv_t[:, ec // 4, kc, (ec % 4) * P : (ec % 4 + 1) * P],
                            hT[:, kc, hf * NQ : (hf + 1) * NQ],
                            start=(kc == 0),
                            stop=(kc == KC - 1),
                        )
                    et = work.tile([P, NQ], BF16, tag="et")
                    nc.scalar.activation(et, ps, AF.Erf, scale=INV_SQRT2)
                    nc.vector.tensor_mul(et, ps, et)
                    nc.vector.tensor_add(
                        vt[:, 1 + hf * NQ : 1 + (hf + 1) * NQ], et, ps
                    )
                bt = work.tile([P, L], BF16, tag="bt")
                for hf in range(2):
                    ps = psum.tile([P, NQ], F32, tag="ps")
                    for kc in range(KC):
                        nc.tensor.matmul(
                            ps,
                            wb_t[:, ec // 4, kc, (ec % 4) * P : (ec % 4 + 1) * P],
                            hT[:, kc, hf * NQ : (hf + 1) * NQ],
                            start=(kc == 0),
                            stop=(kc == KC - 1),
                        )
                    nc.scalar.activation(
                        bt[:, hf * NQ : (hf + 1) * NQ], ps, AF.Sigmoid
                    )
                # beta' = (0.9*sig + 0.1)/2 ; the 1/2 undoes g = 2*gelu(v)
                nc.vector.tensor_scalar(bt, bt, 0.45, 0.05, op0=ALU.mult, op1=ALU.add)
                cv = work.tile([P, L], BF16, tag="cv")
                nc.vector.tensor_scalar_mul(cv, vt[:, 0:L], cwv[:, 0, ec : ec + 1])
                nc.vector.scalar_tensor_tensor(
                    cv, vt[:, 1 : L + 1], cwv[:, 1, ec : ec + 1], cv,
                    op0=ALU.mult, op1=ALU.add,
                )
                nc.vector.scalar_tensor_tensor(
                    cv, vt[:, 2 : L + 2], cwv[:, 2, ec : ec + 1], cv,
                    op0=ALU.mult, op1=ALU.add,
                )
                nc.vector.tensor_mul(cv, cv, bt)
                nc.sync.dma_start_transpose(v_new[:, :, ec * P : (ec + 1) * P], cv)
            # prefetch wout + w1a into the slots that free at P1v's end
            wo_t = wts.tile([P, 2, EC, NQ], BF16, tag="s1")
            w1a_t = wts.tile([P, 4, KC, NQ], BF16, tag="s2")
            for n in range(2):
                nc.scalar.dma_start(wo_t[:, n], wo_d[:, n])
            for n in range(4):
                nc.scalar.dma_start(w1a_t[:, n], w1a_d[:, n])

            if stop_after == "v_new":
                dump3(v_new[:, :, 0:1024])
                return nc

            # ---------------- P6: attn = A @ v_new, fused LN2 stats ----------
            z2T = acts.tile([P, EC, L], BF16, tag="big1")
            for lc in range(LC):
                atn = acts.tile([P, E], BF16, tag="atn", bufs=1)
                ss = work.tile([P, 4], F32, tag="st4", bufs=4)
                sq = work.tile([P, 4], F32, tag="st4", bufs=4)
                for f in range(4):
                    ps = psum.tile([P, NQ], F32, tag="ps")
                    for lpc in range(LC):
                        nc.tensor.matmul(
                            ps,
                            AT[:, lpc, lc * P : (lc + 1) * P],
                            v_new[:, lpc, f * NQ : (f + 1) * NQ],
                            start=(lpc == 0),
                            stop=(lpc == LC - 1),
                        )
                    if attn_scale == 1.0:
                        nc.scalar.activation(
                            atn[:, f * NQ : (f + 1) * NQ], ps, AF.Copy,
                            accum_out=ss[:, f : f + 1],
                        )
                    else:
                        nc.scalar.activation(
                            atn[:, f * NQ : (f + 1) * NQ], ps, AF.Copy,
                            scale=float(attn_scale), accum_out=ss[:, f : f + 1],
                        )
                    nc.scalar.activation(
                        dead1.broadcast_to((P, NQ)), ps, AF.Square,
                        accum_out=sq[:, f : f + 1],
                    )
                s1t = work.tile([P, 1], F32, tag="st1", bufs=8)
                nc.vector.reduce_sum(s1t, ss, axis=AX.X)
                q1t = work.tile([P, 1], F32, tag="st1", bufs=8)
                nc.vector.reduce_sum(q1t, sq, axis=AX.X)
                mean, rstd = rstd_from_sums(s1t, q1t, E, scl=float(attn_scale))
                nc.vector.tensor_scalar(
                    atn, atn, mean, rstd, op0=ALU.subtract, op1=ALU.mult
                )
                nc.sync.dma_start_transpose(z2T[:, :, lc * P : (lc + 1) * P], atn)

            if stop_after == "z2T":
                dump3(z2T[:, 0:8, :])
                return nc

            # ---------------- P8: proj_out + residual -> xnew (SBUF, bf16) ---
            # fused: LN1(xnew) stats accumulate in the evacuations; h2T
            # transposes happen per-lc right here (old P9).
            xnew = acts.tile([P, LC, H], BF16, tag="big3")
            h2T = acts.tile([P, KC, L], BF16, tag="hT")
            for lc in range(LC):
                xt = work.tile([P, H], F32, tag="xy")
                nc.sync.dma_start(xt, x_d[:, lc, :])
                xs = work.tile([P, 2], F32, tag="st4", bufs=4)
                xq = work.tile([P, 2], F32, tag="st4", bufs=4)
                for hc in range(2):
                    ps = psum.tile([P, NQ], F32, tag="ps")
                    for ec in range(EC):
                        nc.tensor.matmul(
                            ps,
                            z2T[:, ec, lc * P : (lc + 1) * P],
                            wo_t[:, hc, ec, :],
                            start=(ec == 0),
                            stop=(ec == EC - 1),
                        )
                    xsl = xnew[:, lc, hc * NQ : (hc + 1) * NQ]
                    nc.vector.scalar_tensor_tensor(
                        xsl, ps, 1.0, xt[:, hc * NQ : (hc + 1) * NQ],
                        op0=ALU.bypass, op1=ALU.add, accum_out=xs[:, hc : hc + 1],
                    )
                    nc.vector.scalar_tensor_tensor(
                        dead1.broadcast_to((P, NQ)), xsl, 1.0, xsl,
                        op0=ALU.bypass, op1=ALU.mult, accum_out=xq[:, hc : hc + 1],
                    )
                s1t = work.tile([P, 1], F32, tag="st1", bufs=8)
                nc.vector.reduce_sum(s1t, xs[:, 0:2], axis=AX.X)
                q1t = work.tile([P, 1], F32, tag="st1", bufs=8)
                nc.vector.reduce_sum(q1t, xq[:, 0:2], axis=AX.X)
                mean, rstd = rstd_from_sums(s1t, q1t, H)
                h2c = work.tile([P, H], BF16, tag="h2c")
                nc.vector.tensor_scalar(
                    h2c, xnew[:, lc, :], mean, rstd,
                    op0=ALU.subtract, op1=ALU.mult,
                )
                nc.sync.dma_start_transpose(h2T[:, :, lc * P : (lc + 1) * P], h2c)
            # w1b into the slot wout frees at P8's end
            w1b_t = wts.tile([P, 4, KC, NQ], BF16, tag="s1")
            for n in range(4):
                nc.scalar.dma_start(w1b_t[:, n], w1b_d[:, n])

            if stop_after == "h2T":
                dump3(h2T)
                return nc

            # ---------------- P10: mlp1 (gelu) -> ug_a, ug_b -----------------
            ug_a = acts.tile([P, JC // 2, L], BF16, tag="big2")
            ug_b = acts.tile([P, JC // 2, L], BF16, tag="big1")
            w2_t = [None, None]
            for half, (w1u, ugx) in enumerate(((w1a_t, ug_a), (w1b_t, ug_b))):
                for jx in range(JC // 2):
                    for hf in range(2):
                        ps = psum.tile([P, NQ], F32, tag="ps")
                        for kc in range(KC):
                            nc.tensor.matmul(
                                ps,
                                w1u[:, jx // 4, kc, (jx % 4) * P : (jx % 4 + 1) * P],
                                h2T[:, kc, hf * NQ : (hf + 1) * NQ],
                                start=(kc == 0),
                                stop=(kc == KC - 1),
                            )
                        nc.scalar.activation(
                            ugx[:, jx, hf * NQ : (hf + 1) * NQ], ps, AF.Gelu
                        )
                # prefetch w2 into the slot this half's w1 frees
                tag = "s2" if half == 0 else "s1"
                w2d = w2a_d if half == 0 else w2b_d
                w2_t[half] = wts.tile(
                    [P, 2, 16, NQ], BF16, tag=tag, name=f"w2_{half}"
                )
                for n in range(2):
                    nc.scalar.dma_start(w2_t[half][:, n], w2d[:, n])

            if stop_after == "ugT":
                dump3(ug_a[:, 0:8, :])
                return nc

            # ---------------- P11: mlp2 + residual -> y ----------------------
            for hc in range(2):
                w2u = w2_t[hc]
                for lc in range(LC):
                    ps = psum.tile([P, NQ], F32, tag="ps")
                    for jc in range(JC):
                        ugx = ug_a if jc < JC // 2 else ug_b
                        nc.tensor.matmul(
                            ps,
                            ugx[:, jc % (JC // 2), lc * P : (lc + 1) * P],
                            w2u[:, jc // 16, jc % 16, :],
                            start=(jc == 0),
                            stop=(jc == JC - 1),
                        )
                    yh = work.tile([P, NQ], F32, tag="yh", bufs=1)
                    nc.vector.tensor_add(
                        yh, ps, xnew[:, lc, hc * NQ : (hc + 1) * NQ]
                    )
                    nc.sync.dma_start(y_d[:, lc, hc * NQ : (hc + 1) * NQ], yh)
    return nc


def _legalize_waits(nc, limit=1):
    """Split excess sync waits onto same-engine NOPs (walrus rejects >limit)."""
    cnt = 0
    for fn in nc.m.functions:
        for bb in fn.blocks:
            insts = bb.instructions
            fixes = []
            for idx, ins in enumerate(insts):
                si = ins.sync_info
                if si is None or not si.on_wait or len(si.on_wait) <= limit:
                    continue
                waits = list(si.on_wait)
                excess, keep = waits[:-limit], waits[-limit:]
                nops = []
                for j in range(0, len(excess), limit):
                    nop = mybir.InstNoOp(name=f"WFIX-{cnt}", text_hint="waitfix")
                    cnt += 1
                    nop.engine = ins.engine
                    nop.sync_info = mybir.SyncInfo(
                        on_wait=excess[j : j + limit], on_update=[]
                    )
                    nops.append(nop)
                si.on_wait = keep
                fixes.append((idx, nops))
            for idx, nops in reversed(fixes):
                for nop in reversed(nops):
                    insts.insert(idx, nop)
    return cnt


def _to_pchunk(a2d, nchunk):
    """[R, C] with R = nchunk*128 -> [128, nchunk, C] (p-major layout)."""
    R, C = a2d.shape
    return np.ascontiguousarray(a2d.reshape(nchunk, P, C).transpose(1, 0, 2))


def _col_chunks(a, nn):
    """[P, KCx, C] -> [P, nn, KCx, C//nn] column-chunk-major."""
    Pp, kk, C = a.shape
    w = C // nn
    return np.ascontiguousarray(
        np.stack([a[:, :, n * w : (n + 1) * w] for n in range(nn)], axis=1)
    )


def _prep_inputs(inputs):
    f32 = lambda a: np.asarray(a, np.float32)
    bf = lambda a: np.ascontiguousarray(a.astype(ml_dtypes.bfloat16))

    x = f32(inputs["x"])
    ln1_w, ln1_b = f32(inputs["ln1_w"]), f32(inputs["ln1_b"])
    ln2_w, ln2_b = f32(inputs["ln2_w"]), f32(inputs["ln2_b"])
    w_qkv, b_qkv = f32(inputs["w_qkv"]), f32(inputs["b_qkv"])
    w_out, b_out = f32(inputs["w_out"]), f32(inputs["b_out"])
    rel_pos = f32(inputs["rel_pos"])
    w_beta, b_beta = f32(inputs["w_beta"]), f32(inputs["b_beta"])
    w1, b1 = f32(inputs["w1"]), f32(inputs["b1"])
    w2, b2 = f32(inputs["w2"]), f32(inputs["b2"])
    conv_w = f32(inputs["conv_w"])
    attn_scale = float(np.asarray(inputs["attn_scale"]).reshape(-1)[0])

    assert not np.any(b_qkv), "nonzero qkv bias not supported"
    assert not np.any(b_out) and not np.any(b2), "nonzero row bias not supported"

    # fold LN affine into the consuming matmuls
    wqkv_e = w_qkv * ln1_w[None, :]
    bqkv_e = b_qkv + w_qkv @ ln1_b
    assert np.allclose(bqkv_e, 0.0), "nonzero folded qkv bias not supported"
    wq_e, wk_e, wv_e = wqkv_e[:E], wqkv_e[E : 2 * E], wqkv_e[2 * E :]

    # beta: comb=[h, pos_info] trick -> rank-1 update, then LN fold
    p_bar = rel_pos[:L].mean(0)
    s = w_beta[:, H:].sum(1)
    wb_raw = w_beta[:, :H] + np.outer(s, p_bar)
    wb_e = wb_raw * ln1_w[None, :]
    bb_e = b_beta + wb_raw @ ln1_b
    assert np.allclose(bb_e, 0.0), "nonzero folded beta bias not supported"

    wout_e = w_out * ln2_w[None, :]
    bout_e = b_out + w_out @ ln2_b
    assert np.allclose(bout_e, 0.0), "nonzero folded out bias not supported"

    w1_e = w1 * ln1_w[None, :]
    b1_e = b1 + w1 @ ln1_b
    assert np.allclose(b1_e, 0.0), "nonzero folded mlp1 bias not supported"

    # conv diag blocks: cd[p, ec, t, m] = conv_w[ec*128+p, 0, t] if p==m else 0
    cd = np.zeros((P, EC, 3, P), np.float32)
    idx = np.arange(P)
    cd[idx, :, :, idx] = conv_w[:, 0, :].reshape(EC, P, 3).transpose(1, 0, 2)
    # vector-conv weights for v: cwv[p, t, ec] = conv_w[ec*128+p, 0, t]
    cwv = np.ascontiguousarray(
        conv_w[:, 0, :].reshape(EC, P, 3).transpose(1, 2, 0)
    )

    w2T = _to_pchunk(w2.T, JC)  # [P, JC, H]

    shared = {
        "wq": bf(_col_chunks(_to_pchunk(wq_e.T, KC), 4)),
        "wk": bf(_col_chunks(_to_pchunk(wk_e.T, KC), 4)),
        "wv": bf(_col_chunks(_to_pchunk(wv_e.T, KC), 4)),
        "wb": bf(_col_chunks(_to_pchunk(wb_e.T, KC), 4)),
        "wo": bf(_col_chunks(_to_pchunk(wout_e.T, EC), 2)),
        "w1a": bf(_col_chunks(_to_pchunk(w1_e.T, KC)[:, :, :E], 4)),
        "w1b": bf(_col_chunks(_to_pchunk(w1_e.T, KC)[:, :, E:], 4)),
        "w2a": bf(np.ascontiguousarray(w2T[:, :, :NQ].reshape(P, 2, 16, NQ))),
        "w2b": bf(np.ascontiguousarray(w2T[:, :, NQ:].reshape(P, 2, 16, NQ))),
        "cdiag": bf(cd),
        "cwv": np.ascontiguousarray(cwv, dtype=np.float32),
    }
    in_maps = []
    for b in range(B):
        m = dict(shared)
        m["x"] = np.ascontiguousarray(x[b].reshape(LC, P, H).transpose(1, 0, 2))
        in_maps.append(m)
    return in_maps, attn_scale


def kernel(**inputs) -> np.ndarray:
    in_maps, attn_scale = _prep_inputs(inputs)
    nc = _build_program(attn_scale)
    _legalize_waits(nc)
    res = run_bass_kernel_spmd(nc, in_maps, core_ids=list(range(B)), trace=TRACE)
    LAST["exec_time_ns"] = res.exec_time_ns
    LAST["results"] = res
    out = np.empty((B, L, H), np.float32)
    for b in range(B):
        yb = np.asarray(res.results[b]["y"])  # [128, LC, H]
        out[b] = yb.transpose(1, 0, 2).reshape(L, H)
    return out


# revision 9
# speedup vs baseline: 1.2984x; 1.0621x over previous
"""DeltaNet block kernel for Trainium2, data-parallel over batch (8 cores).

v3 strategy (per core, one batch element; L=1024, H=1024, E=2048):
  - LN affine params and the pos_embed rank-1 term folded into effective
    weights on the host (exact algebra); all row biases are exactly zero for
    this problem (asserted).
  - Delta-rule einsums in attention form: out = (q @ k^T) @ (beta*v).
  - All matmuls bf16 with fp32 PSUM accumulation; x is carried in bf16.
  - Activation-table discipline: scalar engine sequence is
    {Sqrt} -> {Silu,Square} -> {Sigmoid,Erf,Copy} -> {Copy,Sqrt} -> {Gelu};
    copy/square live in every table set, so ~5 table loads total.  gelu for
    v is computed via erf (sigmoid set): g = x*(erf(x/sqrt2)+1) = 2*gelu(x),
    with the 1/2 folded into the beta gate (beta' = 0.45*sig + 0.05).
  - q/k row-norm rsqrt via bit-trick + 2 Newton steps on the vector engine,
    both rows packed in one [P,2] chain.
  - Depthwise conv(k=3): q/k as 3 accumulating diagonal matmuls on the PE
    (diag blocks DMA'd per-chunk on the gpsimd SWDGE queue); v as 3 fused
    scalar_tensor_tensor taps on the vector engine over a zero-guarded row,
    with the beta gate and final multiply on gpsimd.
  - LN2 / pre-MLP LN1 statistics accumulate during psum evacuations
    (activation accum_out / stt accum_out), collapsing the LN phases.
  - x + attn_out residual (xnew) stays in SBUF in bf16.
  - Weight DMAs are column-chunked and rotate through two 32KB SBUF slots
    (s1: wq->wv->wout->w1b->w2b, s2: wk->wb->w1a->w2a); triggers are emitted
    right after the previous tenant's last compute use so transfers hide
    under compute.
  - hT and h2T are split into column-half tiles so consumers only wait on
    the transposes they actually need (xbar-transpose writes are tracked
    coarsely per tile).
"""

import sys

import numpy as np

sys.path.insert(0, "/opt/trn_rl_repo")

import ml_dtypes  # noqa: E402

import concourse.bass as bass  # noqa: E402
import concourse.mybir as mybir  # noqa: E402
import concourse.tile as tile  # noqa: E402
from concourse.bass_utils import run_bass_kernel_spmd  # noqa: E402

BF16 = mybir.dt.bfloat16
F32 = mybir.dt.float32
I32 = mybir.dt.int32
AF = mybir.ActivationFunctionType
ALU = mybir.AluOpType
AX = mybir.AxisListType

B, L, H, E = 8, 1024, 1024, 2048
P = 128
LC = L // P    # 8
KC = H // P    # 8
EC = E // P    # 16
JC = 4 * H // P  # 32
NQ = 512
EPS = 1e-5
RSQRT_MAGIC = 0x5F3759DF
INV_SQRT2 = 0.7071067811865476

TRACE = False
LAST = {}


def _build_program(attn_scale: float, stop_after: str | None = None):
    nc = bass.Bass("TRN2", target_bir_lowering=False)

    x_d = nc.dram_tensor("x", [P, LC, H], BF16, kind="ExternalInput")
    wq_d = nc.dram_tensor("wq", [P, 4, KC, NQ], BF16, kind="ExternalInput")
    wk_d = nc.dram_tensor("wk", [P, 4, KC, NQ], BF16, kind="ExternalInput")
    wv_d = nc.dram_tensor("wv", [P, 4, KC, NQ], BF16, kind="ExternalInput")
    wb_d = nc.dram_tensor("wb", [P, 4, KC, NQ], BF16, kind="ExternalInput")
    wo_d = nc.dram_tensor("wo", [P, 2, EC, NQ], BF16, kind="ExternalInput")
    w1a_d = nc.dram_tensor("w1a", [P, 4, KC, NQ], BF16, kind="ExternalInput")
    w1b_d = nc.dram_tensor("w1b", [P, 4, KC, NQ], BF16, kind="ExternalInput")
    w2a_d = nc.dram_tensor("w2a", [P, 2, 16, NQ], BF16, kind="ExternalInput")
    w2b_d = nc.dram_tensor("w2b", [P, 2, 16, NQ], BF16, kind="ExternalInput")
    cdiag_d = nc.dram_tensor("cdiag", [P, EC, 3, P], BF16, kind="ExternalInput")
    cwv_d = nc.dram_tensor("cwv", [P, 3, EC], F32, kind="ExternalInput")
    y_d = nc.dram_tensor("y", [P, LC, H], F32, kind="ExternalOutput")

    with tile.TileContext(nc) as tc:
        with (
            tc.tile_pool(name="consts", bufs=1) as consts,
            tc.tile_pool(name="wts", bufs=1) as wts,
            tc.tile_pool(name="acts", bufs=1) as acts,
            tc.tile_pool(name="work", bufs=2) as work,
            tc.tile_pool(name="psum", bufs=8, space="PSUM") as psum,
        ):
            zero_t = consts.tile([P, 1], F32)
            nc.vector.memset(zero_t, 0.0)
            nc.const_aps.aps[(F32, 0.0)] = zero_t[:]
            eps_t = consts.tile([P, 1], F32)
            nc.vector.memset(eps_t, EPS)
            c15_t = consts.tile([P, 2], F32)
            nc.vector.memset(c15_t, 1.5)
            cwv = consts.tile([P, 3, EC], F32)
            nc.sync.dma_start(cwv, cwv_d[:])
            dead1 = consts.tile([P, 1], BF16)  # scalar-engine dead store
            dead2 = consts.tile([P, 1], BF16)  # vector-engine dead store

            def rsqrt2_vec(ssq2):
                """[P,2] f32 sums-of-squares -> [P,2] f32 rsqrt (bit trick +
                2 Newton steps), both lanes in one chain."""
                se = work.tile([P, 2], F32, tag="nt", bufs=8)
                nc.vector.tensor_scalar_add(se, ssq2, 1e-20)
                hh = work.tile([P, 2], F32, tag="nt", bufs=8)
                nc.vector.tensor_scalar_mul(hh, se, -0.5)
                r = work.tile([P, 2], F32, tag="nt", bufs=8)
                nc.vector.tensor_scalar(
                    r.bitcast(I32), se.bitcast(I32), 1, -1,
                    op0=ALU.arith_shift_right, op1=ALU.bitwise_xor,
                )
                nc.vector.tensor_scalar_add(
                    r.bitcast(I32), r.bitcast(I32), RSQRT_MAGIC + 1
                )
                for _ in range(2):
                    yy = work.tile([P, 2], F32, tag="nt", bufs=8)
                    nc.vector.tensor_mul(yy, r, r)
                    ww = work.tile([P, 2], F32, tag="nt", bufs=8)
                    nc.vector.tensor_mul(ww, yy, hh)
                    nc.vector.tensor_add(ww, ww, c15_t)
                    r2 = work.tile([P, 2], F32, tag="nt", bufs=8)
                    nc.vector.tensor_mul(r2, ww, r)
                    r = r2
                return r

            def rstd_from_sums(ssum, sqsum, n, scl=1.0):
                """[P,1] sums of x (scaled by scl) and x^2 (raw) -> mean, rstd."""
                mean = work.tile([P, 1], F32, tag="st1", bufs=8)
                nc.vector.tensor_scalar_mul(mean, ssum, 1.0 / n)
                ex2 = work.tile([P, 1], F32, tag="st1", bufs=8)
                nc.vector.tensor_scalar_mul(ex2, sqsum, scl * scl / n)
                var = work.tile([P, 1], F32, tag="st1", bufs=8)
                nc.vector.tensor_mul(var, mean, mean)
                nc.vector.tensor_sub(var, ex2, var)
                r = work.tile([P, 1], F32, tag="st1", bufs=8)
                nc.scalar.activation(r, var, AF.Sqrt, bias=eps_t[:])
                nc.vector.reciprocal(r, r)
                return mean, r

            def conv3(ps, row, hf, dg):
                """3-tap PE conv into psum ps [P,NQ]; row [P,L] one e-chunk."""
                base = hf * NQ
                nc.tensor.matmul(
                    ps, dg[:, 1, :], row[:, base : base + NQ],
                    start=True, stop=False,
                )
                if hf == 0:
                    nc.tensor.matmul(
                        ps[:, 1:NQ], dg[:, 0, :], row[:, 0 : NQ - 1],
                        start=False, stop=False, skip_group_check=True,
                    )
                    nc.tensor.matmul(
                        ps, dg[:, 2, :], row[:, 1 : NQ + 1],
                        start=False, stop=True, skip_group_check=True,
                    )
                else:
                    nc.tensor.matmul(
                        ps[:, 0 : NQ - 1], dg[:, 2, :], row[:, base + 1 : L],
                        start=False, stop=False, skip_group_check=True,
                    )
                    nc.tensor.matmul(
                        ps, dg[:, 0, :], row[:, base - 1 : base - 1 + NQ],
                        start=False, stop=True, skip_group_check=True,
                    )

            def dump3(src_ap):
                wdt = src_ap.shape[-1]
                for c in range(src_ap.shape[1]):
                    tmp = work.tile([P, H], F32, tag="dbg")
                    nc.vector.tensor_copy(tmp[:, 0:wdt], src_ap[:, c, :])
                    nc.sync.dma_start(y_d[:, c, 0:wdt], tmp[:, 0:wdt])

            # ---------------- P0: LN1(x) -> hTa/hTb [P, KC, NQ] bf16 ---------
            hTa = acts.tile([P, KC, NQ], BF16, tag="hta")
            hTb = acts.tile([P, KC, NQ], BF16, tag="htb")
            for lc in range(LC):
                xt = work.tile([P, H], BF16, tag="xy")
                nc.sync.dma_start(xt, x_d[:, lc, :])
                bnst = work.tile([P, 2, 6], F32, tag="bnst")
                x3 = xt.rearrange("p (s f) -> p s f", s=2)
                for s in range(2):
                    nc.vector.bn_stats(bnst[:, s, :], x3[:, s, :])
                mv = work.tile([P, 2], F32, tag="mv")
                nc.vector.bn_aggr(mv, bnst)
                rstd = work.tile([P, 1], F32, tag="st1", bufs=8)
                nc.scalar.activation(rstd, mv[:, 1:2], AF.Sqrt, bias=eps_t[:])
                nc.vector.reciprocal(rstd, rstd)
                z = work.tile([P, H], BF16, tag="row2k")
                nc.vector.tensor_scalar(
                    z, xt, mv[:, 0:1], rstd, op0=ALU.subtract, op1=ALU.mult
                )
                hdst = hTa if lc < 4 else hTb
                nc.sync.dma_start_transpose(
                    hdst[:, :, (lc % 4) * P : (lc % 4 + 1) * P], z
                )

            def hT_st(kc, lc):
                """stationary hT block [P,P] for token block lc."""
                src = hTa if lc < 4 else hTb
                return src[:, kc, (lc % 4) * P : (lc % 4 + 1) * P]

            if stop_after == "h":
                dump3(hTa)
                return nc

            # ---------------- P3: q,k + silu + normalize-mix -> qT,kT --------
            qT = acts.tile([P, EC, L], BF16, tag="big1")
            kT = acts.tile([P, EC, L], BF16, tag="big2")
            wq_t = wts.tile([P, 4, KC, NQ], BF16, tag="s1")
            wk_t = wts.tile([P, 4, KC, NQ], BF16, tag="s2")
            for n in range(4):
                nc.scalar.dma_start(wq_t[:, n], wq_d[:, n])
            for n in range(4):
                nc.scalar.dma_start(wk_t[:, n], wk_d[:, n])
            for lc in range(LC):
                qs = work.tile([P, E], BF16, tag="qk", bufs=5)
                ks = work.tile([P, E], BF16, tag="qk", bufs=5)
                ssq2 = work.tile([P, 2], F32, tag="st4", bufs=4)
                for wu, dst, col in ((wq_t, qs, 0), (wk_t, ks, 1)):
                    for n in range(4):
                        ps = psum.tile([P, NQ], F32, tag="ps")
                        for kc in range(KC):
                            nc.tensor.matmul(
                                ps,
                                hT_st(kc, lc),
                                wu[:, n, kc, :],
                                start=(kc == 0),
                                stop=(kc == KC - 1),
                            )
                        nc.scalar.activation(dst[:, n * NQ : (n + 1) * NQ], ps, AF.Silu)
                    nc.scalar.activation(
                        dead1.broadcast_to((P, E)), dst, AF.Square,
                        accum_out=ssq2[:, col : col + 1],
                    )
                r2 = rsqrt2_vec(ssq2)
                s = work.tile([P, E], BF16, tag="qk", bufs=5)
                nc.vector.tensor_scalar_mul(s, ks, 0.1)
                nc.vector.scalar_tensor_tensor(
                    qs, qs, r2[:, 0:1], s, op0=ALU.mult, op1=ALU.add
                )
                nc.sync.dma_start_transpose(qT[:, :, lc * P : (lc + 1) * P], qs)
                nc.vector.tensor_scalar_mul(s, qs, 0.1)
                nc.vector.scalar_tensor_tensor(
                    ks, ks, r2[:, 1:2], s, op0=ALU.mult, op1=ALU.add
                )
                nc.sync.dma_start_transpose(kT[:, :, lc * P : (lc + 1) * P], ks)

            if stop_after == "qT":
                dump3(qT[:, 0:8, :])
                return nc

            # ---------------- P4: conv q,k in place (diag matmuls) -----------
            # wv/wb triggers first: WAR on wq/wk (free at P3's last matmul),
            # so they fire immediately and stream during P4+P5.
            wv_t = wts.tile([P, 4, KC, NQ], BF16, tag="s1")
            wb_t = wts.tile([P, 4, KC, NQ], BF16, tag="s2")
            for n in range(4):
                nc.scalar.dma_start(wv_t[:, n], wv_d[:, n])
            for n in range(4):
                nc.scalar.dma_start(wb_t[:, n], wb_d[:, n])
            # conv diag blocks stream on the gpsimd SWDGE queue (depth-2 ahead)
            cdgs = {}
            for e2 in range(2):
                cdg = work.tile([P, 3, P], BF16, tag="cdg", bufs=3, name=f"cdg{e2}")
                nc.gpsimd.dma_start(cdg, cdiag_d[:, e2])
                cdgs[e2] = cdg
            for ec in range(EC):
                if ec + 2 < EC:
                    cdg = work.tile(
                        [P, 3, P], BF16, tag="cdg", bufs=3, name=f"cdg{ec + 2}"
                    )
                    nc.gpsimd.dma_start(cdg, cdiag_d[:, ec + 2])
                    cdgs[ec + 2] = cdg
                dg = cdgs.pop(ec)
                for tz in (qT, kT):
                    ps0 = psum.tile([P, NQ], F32, tag="ps")
                    conv3(ps0, tz[:, ec, :], 0, dg)
                    ps1 = psum.tile([P, NQ], F32, tag="ps")
                    conv3(ps1, tz[:, ec, :], 1, dg)
                    nc.scalar.copy(tz[:, ec, 0:NQ], ps0)
                    nc.scalar.copy(tz[:, ec, NQ : 2 * NQ], ps1)

            if stop_after == "qTc":
                dump3(qT[:, 0:8, :])
                return nc

            # ---------------- P5: AT = (q @ k^T)^T chunks --------------------
            AT = acts.tile([P, LC, L], BF16, tag="big3")
            for lpc in range(LC):
                for hf in range(2):
                    ps = psum.tile([P, NQ], F32, tag="ps")
                    for ec in range(EC):
                        nc.tensor.matmul(
                            ps,
                            kT[:, ec, lpc * P : (lpc + 1) * P],
                            qT[:, ec, hf * NQ : (hf + 1) * NQ],
                            start=(ec == 0),
                            stop=(ec == EC - 1),
                        )
                    nc.scalar.copy(AT[:, lpc, hf * NQ : (hf + 1) * NQ], ps)

            if stop_after == "AT":
                dump3(AT)
                return nc

            # ---------------- P1v: v,beta + erf-gelu + vec conv + transpose --
            v_new = acts.tile([P, LC, E], BF16, tag="big2")
            for ec in range(EC):
                vt = work.tile([P, L + 2], BF16, tag="vt")
                nc.vector.memset(vt[:, 0:1], 0.0)
                nc.vector.memset(vt[:, L + 1 : L + 2], 0.0)
                for hf in range(2):
                    ps = psum.tile([P, NQ], F32, tag="ps")
                    for kc in range(KC):
                        nc.tensor.matmul(
                            ps,
                            wv_t[:, ec // 4, kc, (ec % 4) * P : (ec % 4 + 1) * P],
                            (hTa if hf == 0 else hTb)[:, kc, :],
                            start=(kc == 0),
                            stop=(kc == KC - 1),
                        )
                    et = work.tile([P, NQ], BF16, tag="et", bufs=1)
                    nc.scalar.activation(et, ps, AF.Erf, scale=INV_SQRT2)
                    # et+1 on scalar (Copy table): g = ps*(erf+1) = 2*gelu(ps)
                    nc.scalar.activation(et, et, AF.Copy, bias=1.0)
                    nc.vector.tensor_mul(
                        vt[:, 1 + hf * NQ : 1 + (hf + 1) * NQ], ps, et
                    )
                bt = work.tile([P, L], BF16, tag="row2k")
                for hf in range(2):
                    ps = psum.tile([P, NQ], F32, tag="ps")
                    for kc in range(KC):
                        nc.tensor.matmul(
                            ps,
                            wb_t[:, ec // 4, kc, (ec % 4) * P : (ec % 4 + 1) * P],
                            (hTa if hf == 0 else hTb)[:, kc, :],
                            start=(kc == 0),
                            stop=(kc == KC - 1),
                        )
                    nc.scalar.activation(
                        bt[:, hf * NQ : (hf + 1) * NQ], ps, AF.Sigmoid
                    )
                # beta' = (0.9*sig + 0.1)/2 ; the 1/2 undoes g = 2*gelu(v)
                nc.gpsimd.tensor_scalar(bt, bt, 0.45, 0.05, op0=ALU.mult, op1=ALU.add)
                cv = work.tile([P, L], BF16, tag="cv", bufs=2)
                nc.vector.tensor_scalar_mul(cv, vt[:, 0:L], cwv[:, 0, ec : ec + 1])
                nc.vector.scalar_tensor_tensor(
                    cv, vt[:, 1 : L + 1], cwv[:, 1, ec : ec + 1], cv,
                    op0=ALU.mult, op1=ALU.add,
                )
                nc.vector.scalar_tensor_tensor(
                    cv, vt[:, 2 : L + 2], cwv[:, 2, ec : ec + 1], cv,
                    op0=ALU.mult, op1=ALU.add,
                )
                nc.gpsimd.tensor_mul(cv, cv, bt)
                nc.sync.dma_start_transpose(v_new[:, :, ec * P : (ec + 1) * P], cv)
            # prefetch wout + w1a into the slots that free at P1v's end
            wo_t = wts.tile([P, 2, EC, NQ], BF16, tag="s1")
            w1a_t = wts.tile([P, 4, KC, NQ], BF16, tag="s2")
            for n in range(2):
                nc.scalar.dma_start(wo_t[:, n], wo_d[:, n])
            for n in range(4):
                nc.scalar.dma_start(w1a_t[:, n], w1a_d[:, n])

            if stop_after == "v_new":
                dump3(v_new[:, :, 0:1024])
                return nc

            # ---------------- P6: attn = A @ v_new, fused LN2 stats ----------
            z2T = acts.tile([P, EC, L], BF16, tag="big1")
            for lc in range(LC):
                atn = acts.tile([P, E], BF16, tag="atn", bufs=1)
                ss = work.tile([P, 4], F32, tag="st4", bufs=4)
                sq = work.tile([P, 4], F32, tag="st4", bufs=4)
                for f in range(4):
                    ps = psum.tile([P, NQ], F32, tag="ps")
                    for lpc in range(LC):
                        nc.tensor.matmul(
                            ps,
                            AT[:, lpc, lc * P : (lc + 1) * P],
                            v_new[:, lpc, f * NQ : (f + 1) * NQ],
                            start=(lpc == 0),
                            stop=(lpc == LC - 1),
                        )
                    if attn_scale == 1.0:
                        nc.scalar.activation(
                            atn[:, f * NQ : (f + 1) * NQ], ps, AF.Copy,
                            accum_out=ss[:, f : f + 1],
                        )
                    else:
                        nc.scalar.activation(
                            atn[:, f * NQ : (f + 1) * NQ], ps, AF.Copy,
                            scale=float(attn_scale), accum_out=ss[:, f : f + 1],
                        )
                    nc.scalar.activation(
                        dead1.broadcast_to((P, NQ)), ps, AF.Square,
                        accum_out=sq[:, f : f + 1],
                    )
                s1t = work.tile([P, 1], F32, tag="st1", bufs=8)
                nc.vector.reduce_sum(s1t, ss, axis=AX.X)
                q1t = work.tile([P, 1], F32, tag="st1", bufs=8)
                nc.vector.reduce_sum(q1t, sq, axis=AX.X)
                mean, rstd = rstd_from_sums(s1t, q1t, E, scl=float(attn_scale))
                nc.vector.tensor_scalar(
                    atn, atn, mean, rstd, op0=ALU.subtract, op1=ALU.mult
                )
                nc.sync.dma_start_transpose(z2T[:, :, lc * P : (lc + 1) * P], atn)

            if stop_after == "z2T":
                dump3(z2T[:, 0:8, :])
                return nc

            # ---------------- P8: proj_out + residual -> xnew (SBUF, bf16) ---
            # LN1(xnew) stats accumulate in the evacuations; h2T transposes
            # happen per-lc right here.
            xnew = acts.tile([P, LC, H], BF16, tag="big3")
            h2Ta = acts.tile([P, KC, NQ], BF16, tag="hta")
            h2Tb = acts.tile([P, KC, NQ], BF16, tag="htb")
            for lc in range(LC):
                xt = work.tile([P, H], BF16, tag="xy")
                nc.sync.dma_start(xt, x_d[:, lc, :])
                xs = work.tile([P, 2], F32, tag="st4", bufs=4)
                xq = work.tile([P, 2], F32, tag="st4", bufs=4)
                for hc in range(2):
                    ps = psum.tile([P, NQ], F32, tag="ps")
                    for ec in range(EC):
                        nc.tensor.matmul(
                            ps,
                            z2T[:, ec, lc * P : (lc + 1) * P],
                            wo_t[:, hc, ec, :],
                            start=(ec == 0),
                            stop=(ec == EC - 1),
                        )
                    xsl = xnew[:, lc, hc * NQ : (hc + 1) * NQ]
                    nc.vector.scalar_tensor_tensor(
                        xsl, ps, 1.0, xt[:, hc * NQ : (hc + 1) * NQ],
                        op0=ALU.bypass, op1=ALU.add, accum_out=xs[:, hc : hc + 1],
                    )
                    nc.vector.scalar_tensor_tensor(
                        dead2.broadcast_to((P, NQ)), xsl, 1.0, xsl,
                        op0=ALU.bypass, op1=ALU.mult, accum_out=xq[:, hc : hc + 1],
                    )
                s1t = work.tile([P, 1], F32, tag="st1", bufs=8)
                nc.vector.reduce_sum(s1t, xs[:, 0:2], axis=AX.X)
                q1t = work.tile([P, 1], F32, tag="st1", bufs=8)
                nc.vector.reduce_sum(q1t, xq[:, 0:2], axis=AX.X)
                mean, rstd = rstd_from_sums(s1t, q1t, H)
                h2c = work.tile([P, H], BF16, tag="row2k")
                nc.vector.tensor_scalar(
                    h2c, xnew[:, lc, :], mean, rstd,
                    op0=ALU.subtract, op1=ALU.mult,
                )
                hdst = h2Ta if lc < 4 else h2Tb
                nc.sync.dma_start_transpose(
                    hdst[:, :, (lc % 4) * P : (lc % 4 + 1) * P], h2c
                )
            # w1b into the slot wout frees at P8's end
            w1b_t = wts.tile([P, 4, KC, NQ], BF16, tag="s1")
            for n in range(4):
                nc.scalar.dma_start(w1b_t[:, n], w1b_d[:, n])

            if stop_after == "h2T":
                dump3(h2Ta)
                return nc

            # ---------------- P10: mlp1 (gelu) -> ug_a, ug_b -----------------
            ug_a = acts.tile([P, JC // 2, L], BF16, tag="big2")
            ug_b = acts.tile([P, JC // 2, L], BF16, tag="big1")
            w2_t = [None, None]
            for half, (w1u, ugx) in enumerate(((w1a_t, ug_a), (w1b_t, ug_b))):
                for hf in range(2):
                    for jx in range(JC // 2):
                        ps = psum.tile([P, NQ], F32, tag="ps")
                        for kc in range(KC):
                            nc.tensor.matmul(
                                ps,
                                w1u[:, jx // 4, kc, (jx % 4) * P : (jx % 4 + 1) * P],
                                (h2Ta if hf == 0 else h2Tb)[:, kc, :],
                                start=(kc == 0),
                                stop=(kc == KC - 1),
                            )
                        nc.scalar.activation(
                            ugx[:, jx, hf * NQ : (hf + 1) * NQ], ps, AF.Gelu
                        )
                # prefetch w2 into the slot this half's w1 frees
                tag = "s2" if half == 0 else "s1"
                w2d = w2a_d if half == 0 else w2b_d
                w2_t[half] = wts.tile(
                    [P, 2, 16, NQ], BF16, tag=tag, name=f"w2_{half}"
                )
                for n in range(2):
                    nc.scalar.dma_start(w2_t[half][:, n], w2d[:, n])

            if stop_after == "ugT":
                dump3(ug_a[:, 0:8, :])
                return nc

            # ---------------- P11: mlp2 + residual -> y ----------------------
            for hc in range(2):
                w2u = w2_t[hc]
                for lc in range(LC):
                    ps = psum.tile([P, NQ], F32, tag="ps")
                    for jc in range(JC):
                        ugx = ug_a if jc < JC // 2 else ug_b
                        nc.tensor.matmul(
                            ps,
                            ugx[:, jc % (JC // 2), lc * P : (lc + 1) * P],
                            w2u[:, jc // 16, jc % 16, :],
                            start=(jc == 0),
                            stop=(jc == JC - 1),
                        )
                    yh = work.tile([P, NQ], F32, tag="yh", bufs=1)
                    nc.vector.tensor_add(
                        yh, ps, xnew[:, lc, hc * NQ : (hc + 1) * NQ]
                    )
                    nc.sync.dma_start(y_d[:, lc, hc * NQ : (hc + 1) * NQ], yh)
    return nc


def _legalize_waits(nc, limit=1):
    """Split excess sync waits onto same-engine NOPs (walrus rejects >limit)."""
    cnt = 0
    for fn in nc.m.functions:
        for bb in fn.blocks:
            insts = bb.instructions
            fixes = []
            for idx, ins in enumerate(insts):
                si = ins.sync_info
                if si is None or not si.on_wait or len(si.on_wait) <= limit:
                    continue
                waits = list(si.on_wait)
                excess, keep = waits[:-limit], waits[-limit:]
                nops = []
                for j in range(0, len(excess), limit):
                    nop = mybir.InstNoOp(name=f"WFIX-{cnt}", text_hint="waitfix")
                    cnt += 1
                    nop.engine = ins.engine
                    nop.sync_info = mybir.SyncInfo(
                        on_wait=excess[j : j + limit], on_update=[]
                    )
                    nops.append(nop)
                si.on_wait = keep
                fixes.append((idx, nops))
            for idx, nops in reversed(fixes):
                for nop in reversed(nops):
                    insts.insert(idx, nop)
    return cnt


def _to_pchunk(a2d, nchunk):
    """[R, C] with R = nchunk*128 -> [128, nchunk, C] (p-major layout)."""
    R, C = a2d.shape
    return np.ascontiguousarray(a2d.reshape(nchunk, P, C).transpose(1, 0, 2))


def _col_chunks(a, nn):
    """[P, KCx, C] -> [P, nn, KCx, C//nn] column-chunk-major."""
    Pp, kk, C = a.shape
    w = C // nn
    return np.ascontiguousarray(
        np.stack([a[:, :, n * w : (n + 1) * w] for n in range(nn)], axis=1)
    )


def _prep_inputs(inputs):
    f32 = lambda a: np.asarray(a, np.float32)
    bf = lambda a: np.ascontiguousarray(a.astype(ml_dtypes.bfloat16))

    x = f32(inputs["x"])
    ln1_w, ln1_b = f32(inputs["ln1_w"]), f32(inputs["ln1_b"])
    ln2_w, ln2_b = f32(inputs["ln2_w"]), f32(inputs["ln2_b"])
    w_qkv, b_qkv = f32(inputs["w_qkv"]), f32(inputs["b_qkv"])
    w_out, b_out = f32(inputs["w_out"]), f32(inputs["b_out"])
    rel_pos = f32(inputs["rel_pos"])
    w_beta, b_beta = f32(inputs["w_beta"]), f32(inputs["b_beta"])
    w1, b1 = f32(inputs["w1"]), f32(inputs["b1"])
    w2, b2 = f32(inputs["w2"]), f32(inputs["b2"])
    conv_w = f32(inputs["conv_w"])
    attn_scale = float(np.asarray(inputs["attn_scale"]).reshape(-1)[0])

    assert not np.any(b_qkv), "nonzero qkv bias not supported"
    assert not np.any(b_out) and not np.any(b2), "nonzero row bias not supported"

    # fold LN affine into the consuming matmuls
    wqkv_e = w_qkv * ln1_w[None, :]
    bqkv_e = b_qkv + w_qkv @ ln1_b
    assert np.allclose(bqkv_e, 0.0), "nonzero folded qkv bias not supported"
    wq_e, wk_e, wv_e = wqkv_e[:E], wqkv_e[E : 2 * E], wqkv_e[2 * E :]

    # beta: comb=[h, pos_info] trick -> rank-1 update, then LN fold
    p_bar = rel_pos[:L].mean(0)
    s = w_beta[:, H:].sum(1)
    wb_raw = w_beta[:, :H] + np.outer(s, p_bar)
    wb_e = wb_raw * ln1_w[None, :]
    bb_e = b_beta + wb_raw @ ln1_b
    assert np.allclose(bb_e, 0.0), "nonzero folded beta bias not supported"

    wout_e = w_out * ln2_w[None, :]
    bout_e = b_out + w_out @ ln2_b
    assert np.allclose(bout_e, 0.0), "nonzero folded out bias not supported"

    w1_e = w1 * ln1_w[None, :]
    b1_e = b1 + w1 @ ln1_b
    assert np.allclose(b1_e, 0.0), "nonzero folded mlp1 bias not supported"

    # conv diag blocks: cd[p, ec, t, m] = conv_w[ec*128+p, 0, t] if p==m else 0
    cd = np.zeros((P, EC, 3, P), np.float32)
    idx = np.arange(P)
    cd[idx, :, :, idx] = conv_w[:, 0, :].reshape(EC, P, 3).transpose(1, 0, 2)
    # vector-conv weights for v: cwv[p, t, ec] = conv_w[ec*128+p, 0, t]
    cwv = np.ascontiguousarray(
        conv_w[:, 0, :].reshape(EC, P, 3).transpose(1, 2, 0)
    )

    w2T = _to_pchunk(w2.T, JC)  # [P, JC, H]

    shared = {
        "wq": bf(_col_chunks(_to_pchunk(wq_e.T, KC), 4)),
        "wk": bf(_col_chunks(_to_pchunk(wk_e.T, KC), 4)),
        "wv": bf(_col_chunks(_to_pchunk(wv_e.T, KC), 4)),
        "wb": bf(_col_chunks(_to_pchunk(wb_e.T, KC), 4)),
        "wo": bf(_col_chunks(_to_pchunk(wout_e.T, EC), 2)),
        "w1a": bf(_col_chunks(_to_pchunk(w1_e.T, KC)[:, :, :E], 4)),
        "w1b": bf(_col_chunks(_to_pchunk(w1_e.T, KC)[:, :, E:], 4)),
        "w2a": bf(np.ascontiguousarray(w2T[:, :, :NQ].reshape(P, 2, 16, NQ))),
        "w2b": bf(np.ascontiguousarray(w2T[:, :, NQ:].reshape(P, 2, 16, NQ))),
        "cdiag": bf(cd),
        "cwv": np.ascontiguousarray(cwv, dtype=np.float32),
    }
    in_maps = []
    for b in range(B):
        m = dict(shared)
        m["x"] = bf(x[b].reshape(LC, P, H).transpose(1, 0, 2))
        in_maps.append(m)
    return in_maps, attn_scale


def kernel(**inputs) -> np.ndarray:
    in_maps, attn_scale = _prep_inputs(inputs)
    nc = _build_program(attn_scale)
    _legalize_waits(nc)
    res = run_bass_kernel_spmd(nc, in_maps, core_ids=list(range(B)), trace=TRACE)
    LAST["exec_time_ns"] = res.exec_time_ns
    LAST["results"] = res
    out = np.empty((B, L, H), np.float32)
    for b in range(B):
        yb = np.asarray(res.results[b]["y"])  # [128, LC, H]
        out[b] = yb.transpose(1, 0, 2).reshape(L, H)
    return out


# revision 10
# speedup vs baseline: 1.3410x; 1.0328x over previous
"""DeltaNet block kernel for Trainium2, data-parallel over batch (8 cores).

v3 strategy (per core, one batch element; L=1024, H=1024, E=2048):
  - LN affine params and the pos_embed rank-1 term folded into effective
    weights on the host (exact algebra); all row biases are exactly zero for
    this problem (asserted).
  - Delta-rule einsums in attention form: out = (q @ k^T) @ (beta*v).
  - All matmuls bf16 with fp32 PSUM accumulation; x is carried in bf16.
  - Activation-table discipline: scalar engine sequence is
    {Sqrt} -> {Silu,Square} -> {Sigmoid,Erf,Copy} -> {Copy,Sqrt} -> {Gelu};
    copy/square live in every table set, so ~5 table loads total.  gelu for
    v is computed via erf (sigmoid set): g = x*(erf(x/sqrt2)+1) = 2*gelu(x),
    with the 1/2 folded into the beta gate (beta' = 0.45*sig + 0.05).
  - q/k row-norm rsqrt via bit-trick + 2 Newton steps on the vector engine,
    both rows packed in one [P,2] chain.
  - Depthwise conv(k=3): q/k as 3 accumulating diagonal matmuls on the PE
    (diag blocks DMA'd per-chunk on the gpsimd SWDGE queue); v as 3 fused
    scalar_tensor_tensor taps on the vector engine over a zero-guarded row,
    with the beta gate and final multiply on gpsimd.
  - LN2 / pre-MLP LN1 statistics accumulate during psum evacuations
    (activation accum_out / stt accum_out), collapsing the LN phases.
  - x + attn_out residual (xnew) stays in SBUF in bf16.
  - Weight DMAs are column-chunked and rotate through two 32KB SBUF slots
    (s1: wq->wv->wout->w1b->w2b, s2: wk->wb->w1a->w2a); triggers are emitted
    right after the previous tenant's last compute use so transfers hide
    under compute.
  - hT and h2T are split into column-half tiles so consumers only wait on
    the transposes they actually need (xbar-transpose writes are tracked
    coarsely per tile).
"""

import sys

import numpy as np

sys.path.insert(0, "/opt/trn_rl_repo")

import ml_dtypes  # noqa: E402

import concourse.bass as bass  # noqa: E402
import concourse.mybir as mybir  # noqa: E402
import concourse.tile as tile  # noqa: E402
from concourse.bass_utils import run_bass_kernel_spmd  # noqa: E402

BF16 = mybir.dt.bfloat16
F32 = mybir.dt.float32
I32 = mybir.dt.int32
AF = mybir.ActivationFunctionType
ALU = mybir.AluOpType
AX = mybir.AxisListType

B, L, H, E = 8, 1024, 1024, 2048
P = 128
LC = L // P    # 8
KC = H // P    # 8
EC = E // P    # 16
JC = 4 * H // P  # 32
NQ = 512
EPS = 1e-5
RSQRT_MAGIC = 0x5F3759DF
INV_SQRT2 = 0.7071067811865476

TRACE = False
LAST = {}


def _build_program(attn_scale: float, stop_after: str | None = None):
    nc = bass.Bass("TRN2", target_bir_lowering=False)

    x_d = nc.dram_tensor("x", [P, LC, H], BF16, kind="ExternalInput")
    wq_d = nc.dram_tensor("wq", [P, 4, KC, NQ], BF16, kind="ExternalInput")
    wk_d = nc.dram_tensor("wk", [P, 4, KC, NQ], BF16, kind="ExternalInput")
    wv_d = nc.dram_tensor("wv", [P, 4, KC, NQ], BF16, kind="ExternalInput")
    wb_d = nc.dram_tensor("wb", [P, 4, KC, NQ], BF16, kind="ExternalInput")
    wo_d = nc.dram_tensor("wo", [P, 2, EC, NQ], BF16, kind="ExternalInput")
    w1a_d = nc.dram_tensor("w1a", [P, 4, KC, NQ], BF16, kind="ExternalInput")
    w1b_d = nc.dram_tensor("w1b", [P, 4, KC, NQ], BF16, kind="ExternalInput")
    w2a_d = nc.dram_tensor("w2a", [P, 2, 16, NQ], BF16, kind="ExternalInput")
    w2b_d = nc.dram_tensor("w2b", [P, 2, 16, NQ], BF16, kind="ExternalInput")
    cdiag_d = nc.dram_tensor("cdiag", [P, EC, 3, P], BF16, kind="ExternalInput")
    cwv_d = nc.dram_tensor("cwv", [P, 3, EC], F32, kind="ExternalInput")
    y_d = nc.dram_tensor("y", [P, LC, H], F32, kind="ExternalOutput")

    with tile.TileContext(nc) as tc:
        with (
            tc.tile_pool(name="consts", bufs=1) as consts,
            tc.tile_pool(name="wts", bufs=1) as wts,
            tc.tile_pool(name="acts", bufs=1) as acts,
            tc.tile_pool(name="work", bufs=2) as work,
            tc.tile_pool(name="psum", bufs=8, space="PSUM") as psum,
        ):
            zero_t = consts.tile([P, 1], F32)
            nc.vector.memset(zero_t, 0.0)
            nc.const_aps.aps[(F32, 0.0)] = zero_t[:]
            eps_t = consts.tile([P, 1], F32)
            nc.vector.memset(eps_t, EPS)
            c15_t = consts.tile([P, 2], F32)
            nc.vector.memset(c15_t, 1.5)
            cwv = consts.tile([P, 3, EC], F32)
            nc.sync.dma_start(cwv, cwv_d[:])
            dead1 = consts.tile([P, 1], BF16)  # scalar-engine dead store
            dead2 = consts.tile([P, 1], BF16)  # vector-engine dead store

            def rsqrt2_vec(ssq2):
                """[P,2] f32 sums-of-squares -> [P,2] f32 rsqrt (bit trick +
                2 Newton steps), both lanes in one chain."""
                se = work.tile([P, 2], F32, tag="nt", bufs=8)
                nc.vector.tensor_scalar_add(se, ssq2, 1e-20)
                hh = work.tile([P, 2], F32, tag="nt", bufs=8)
                nc.vector.tensor_scalar_mul(hh, se, -0.5)
                r = work.tile([P, 2], F32, tag="nt", bufs=8)
                nc.vector.tensor_scalar(
                    r.bitcast(I32), se.bitcast(I32), 1, -1,
                    op0=ALU.arith_shift_right, op1=ALU.bitwise_xor,
                )
                nc.vector.tensor_scalar_add(
                    r.bitcast(I32), r.bitcast(I32), RSQRT_MAGIC + 1
                )
                for _ in range(2):
                    yy = work.tile([P, 2], F32, tag="nt", bufs=8)
                    nc.vector.tensor_mul(yy, r, r)
                    ww = work.tile([P, 2], F32, tag="nt", bufs=8)
                    nc.vector.tensor_mul(ww, yy, hh)
                    nc.vector.tensor_add(ww, ww, c15_t)
                    r2 = work.tile([P, 2], F32, tag="nt", bufs=8)
                    nc.vector.tensor_mul(r2, ww, r)
                    r = r2
                return r

            def rstd_from_sums(ssum, sqsum, n, scl=1.0):
                """[P,1] sums of x (scaled by scl) and x^2 (raw) -> mean, rstd."""
                mean = work.tile([P, 1], F32, tag="st1", bufs=8)
                nc.vector.tensor_scalar_mul(mean, ssum, 1.0 / n)
                ex2 = work.tile([P, 1], F32, tag="st1", bufs=8)
                nc.vector.tensor_scalar_mul(ex2, sqsum, scl * scl / n)
                var = work.tile([P, 1], F32, tag="st1", bufs=8)
                nc.vector.tensor_mul(var, mean, mean)
                nc.vector.tensor_sub(var, ex2, var)
                r = work.tile([P, 1], F32, tag="st1", bufs=8)
                nc.scalar.activation(r, var, AF.Sqrt, bias=eps_t[:])
                nc.vector.reciprocal(r, r)
                return mean, r

            def conv3(ps, row, hf, dg):
                """3-tap PE conv into psum ps [P,NQ]; row [P,L] one e-chunk."""
                base = hf * NQ
                nc.tensor.matmul(
                    ps, dg[:, 1, :], row[:, base : base + NQ],
                    start=True, stop=False,
                )
                if hf == 0:
                    nc.tensor.matmul(
                        ps[:, 1:NQ], dg[:, 0, :], row[:, 0 : NQ - 1],
                        start=False, stop=False, skip_group_check=True,
                    )
                    nc.tensor.matmul(
                        ps, dg[:, 2, :], row[:, 1 : NQ + 1],
                        start=False, stop=True, skip_group_check=True,
                    )
                else:
                    nc.tensor.matmul(
                        ps[:, 0 : NQ - 1], dg[:, 2, :], row[:, base + 1 : L],
                        start=False, stop=False, skip_group_check=True,
                    )
                    nc.tensor.matmul(
                        ps, dg[:, 0, :], row[:, base - 1 : base - 1 + NQ],
                        start=False, stop=True, skip_group_check=True,
                    )

            def dump3(src_ap):
                wdt = src_ap.shape[-1]
                for c in range(src_ap.shape[1]):
                    tmp = work.tile([P, H], F32, tag="dbg")
                    nc.vector.tensor_copy(tmp[:, 0:wdt], src_ap[:, c, :])
                    nc.sync.dma_start(y_d[:, c, 0:wdt], tmp[:, 0:wdt])

            # ---------------- P0: LN1(x) -> hTa/hTb [P, KC, NQ] bf16 ---------
            hTa = acts.tile([P, KC, NQ], BF16, tag="hta")
            hTb = acts.tile([P, KC, NQ], BF16, tag="htb")
            for lc in range(LC):
                xt = work.tile([P, H], BF16, tag="xy")
                nc.sync.dma_start(xt, x_d[:, lc, :])
                bnst = work.tile([P, 2, 6], F32, tag="bnst")
                x3 = xt.rearrange("p (s f) -> p s f", s=2)
                for s in range(2):
                    nc.vector.bn_stats(bnst[:, s, :], x3[:, s, :])
                mv = work.tile([P, 2], F32, tag="mv")
                nc.vector.bn_aggr(mv, bnst)
                rstd = work.tile([P, 1], F32, tag="st1", bufs=8)
                nc.scalar.activation(rstd, mv[:, 1:2], AF.Sqrt, bias=eps_t[:])
                nc.vector.reciprocal(rstd, rstd)
                z = work.tile([P, H], BF16, tag="row2k")
                nc.vector.tensor_scalar(
                    z, xt, mv[:, 0:1], rstd, op0=ALU.subtract, op1=ALU.mult
                )
                hdst = hTa if lc < 4 else hTb
                nc.sync.dma_start_transpose(
                    hdst[:, :, (lc % 4) * P : (lc % 4 + 1) * P], z
                )

            def hT_st(kc, lc):
                """stationary hT block [P,P] for token block lc."""
                src = hTa if lc < 4 else hTb
                return src[:, kc, (lc % 4) * P : (lc % 4 + 1) * P]

            if stop_after == "h":
                dump3(hTa)
                return nc

            # ---------------- P3: q,k + silu + normalize-mix -> qT,kT --------
            qT = acts.tile([P, EC, L], BF16, tag="big1")
            kT = acts.tile([P, EC, L], BF16, tag="big2")
            wq_t = wts.tile([P, 4, KC, NQ], BF16, tag="s1")
            wk_t = wts.tile([P, 4, KC, NQ], BF16, tag="s2")
            for n in range(4):
                nc.scalar.dma_start(wq_t[:, n], wq_d[:, n])
            for n in range(4):
                nc.scalar.dma_start(wk_t[:, n], wk_d[:, n])
            for lc in range(LC):
                qs = work.tile([P, E], BF16, tag="qk", bufs=4)
                ks = work.tile([P, E], BF16, tag="qk", bufs=4)
                sq4 = work.tile([P, 4], F32, tag="st4", bufs=4)
                sk4 = work.tile([P, 4], F32, tag="st4", bufs=4)
                for wu, dst, acc4 in ((wq_t, qs, sq4), (wk_t, ks, sk4)):
                    for n in range(4):
                        ps = psum.tile([P, NQ], F32, tag="ps")
                        for kc in range(KC):
                            nc.tensor.matmul(
                                ps,
                                hT_st(kc, lc),
                                wu[:, n, kc, :],
                                start=(kc == 0),
                                stop=(kc == KC - 1),
                            )
                        chunk = dst[:, n * NQ : (n + 1) * NQ]
                        nc.scalar.activation(chunk, ps, AF.Silu)
                        nc.vector.scalar_tensor_tensor(
                            dead2.broadcast_to((P, NQ)), chunk, 1.0, chunk,
                            op0=ALU.bypass, op1=ALU.mult,
                            accum_out=acc4[:, n : n + 1],
                        )
                ssq2 = work.tile([P, 2], F32, tag="st4", bufs=4)
                nc.vector.reduce_sum(ssq2[:, 0:1], sq4, axis=AX.X)
                nc.vector.reduce_sum(ssq2[:, 1:2], sk4, axis=AX.X)
                r2 = rsqrt2_vec(ssq2)
                s = work.tile([P, E], BF16, tag="qk", bufs=4)
                nc.vector.tensor_scalar_mul(s, ks, 0.1)
                nc.vector.tensor_scalar_mul(qs, qs, r2[:, 0:1])
                nc.vector.tensor_add(qs, qs, s)
                nc.sync.dma_start_transpose(qT[:, :, lc * P : (lc + 1) * P], qs)
                nc.vector.tensor_scalar_mul(s, qs, 0.1)
                nc.vector.tensor_scalar_mul(ks, ks, r2[:, 1:2])
                nc.vector.tensor_add(ks, ks, s)
                nc.sync.dma_start_transpose(kT[:, :, lc * P : (lc + 1) * P], ks)

            if stop_after == "qT":
                dump3(qT[:, 0:8, :])
                return nc

            # ---------------- P4: conv q,k in place (diag matmuls) -----------
            # wv/wb triggers first: WAR on wq/wk (free at P3's last matmul),
            # so they fire immediately and stream during P4+P5.
            wv_t = wts.tile([P, 4, KC, NQ], BF16, tag="s1")
            wb_t = wts.tile([P, 4, KC, NQ], BF16, tag="s2")
            for n in range(4):
                nc.scalar.dma_start(wv_t[:, n], wv_d[:, n])
            for n in range(4):
                nc.scalar.dma_start(wb_t[:, n], wb_d[:, n])
            # conv diag blocks stream on the gpsimd SWDGE queue (depth-2 ahead)
            cdgs = {}
            for e2 in range(2):
                cdg = work.tile([P, 3, P], BF16, tag="cdg", bufs=2, name=f"cdg{e2}")
                nc.gpsimd.dma_start(cdg, cdiag_d[:, e2])
                cdgs[e2] = cdg
            for ec in range(EC):
                if ec + 2 < EC:
                    cdg = work.tile(
                        [P, 3, P], BF16, tag="cdg", bufs=2, name=f"cdg{ec + 2}"
                    )
                    nc.gpsimd.dma_start(cdg, cdiag_d[:, ec + 2])
                    cdgs[ec + 2] = cdg
                dg = cdgs.pop(ec)
                for tz in (qT, kT):
                    ps0 = psum.tile([P, NQ], F32, tag="ps")
                    conv3(ps0, tz[:, ec, :], 0, dg)
                    ps1 = psum.tile([P, NQ], F32, tag="ps")
                    conv3(ps1, tz[:, ec, :], 1, dg)
                    nc.scalar.copy(tz[:, ec, 0:NQ], ps0)
                    nc.scalar.copy(tz[:, ec, NQ : 2 * NQ], ps1)

            if stop_after == "qTc":
                dump3(qT[:, 0:8, :])
                return nc

            # ---------------- P5: AT = (q @ k^T)^T chunks --------------------
            AT = acts.tile([P, LC, L], BF16, tag="big3")
            for lpc in range(LC):
                for hf in range(2):
                    ps = psum.tile([P, NQ], F32, tag="ps")
                    for ec in range(EC):
                        nc.tensor.matmul(
                            ps,
                            kT[:, ec, lpc * P : (lpc + 1) * P],
                            qT[:, ec, hf * NQ : (hf + 1) * NQ],
                            start=(ec == 0),
                            stop=(ec == EC - 1),
                        )
                    nc.scalar.copy(AT[:, lpc, hf * NQ : (hf + 1) * NQ], ps)

            if stop_after == "AT":
                dump3(AT)
                return nc

            # ---------------- P1v: v,beta + erf-gelu + vec conv + transpose --
            v_new = acts.tile([P, LC, E], BF16, tag="big2")
            for ec in range(EC):
                vt = work.tile([P, L + 2], BF16, tag="vt")
                nc.vector.memset(vt[:, 0:1], 0.0)
                nc.vector.memset(vt[:, L + 1 : L + 2], 0.0)
                bt = work.tile([P, L], BF16, tag="row2k")
                for hf in range(2):
                    ps = psum.tile([P, NQ], F32, tag="ps")
                    for kc in range(KC):
                        nc.tensor.matmul(
                            ps,
                            wb_t[:, ec // 4, kc, (ec % 4) * P : (ec % 4 + 1) * P],
                            (hTa if hf == 0 else hTb)[:, kc, :],
                            start=(kc == 0),
                            stop=(kc == KC - 1),
                        )
                    nc.scalar.activation(
                        bt[:, hf * NQ : (hf + 1) * NQ], ps, AF.Sigmoid
                    )
                # beta' = (0.9*sig + 0.1)/2 ; the 1/2 undoes g = 2*gelu(v)
                # (overlaps the v matmuls below on the gpsimd engine)
                nc.gpsimd.tensor_scalar(bt, bt, 0.45, 0.05, op0=ALU.mult, op1=ALU.add)
                for hf in range(2):
                    ps = psum.tile([P, NQ], F32, tag="ps")
                    for kc in range(KC):
                        nc.tensor.matmul(
                            ps,
                            wv_t[:, ec // 4, kc, (ec % 4) * P : (ec % 4 + 1) * P],
                            (hTa if hf == 0 else hTb)[:, kc, :],
                            start=(kc == 0),
                            stop=(kc == KC - 1),
                        )
                    et = work.tile([P, NQ], BF16, tag="et", bufs=1)
                    nc.scalar.activation(et, ps, AF.Erf, scale=INV_SQRT2)
                    # et+1 on scalar (Copy table): g = ps*(erf+1) = 2*gelu(ps)
                    nc.scalar.activation(et, et, AF.Copy, bias=1.0)
                    nc.vector.tensor_mul(
                        vt[:, 1 + hf * NQ : 1 + (hf + 1) * NQ], ps, et
                    )
                cv = work.tile([P, L], BF16, tag="cv", bufs=2)
                nc.vector.tensor_scalar_mul(cv, vt[:, 0:L], cwv[:, 0, ec : ec + 1])
                nc.vector.scalar_tensor_tensor(
                    cv, vt[:, 1 : L + 1], cwv[:, 1, ec : ec + 1], cv,
                    op0=ALU.mult, op1=ALU.add,
                )
                nc.vector.scalar_tensor_tensor(
                    cv, vt[:, 2 : L + 2], cwv[:, 2, ec : ec + 1], cv,
                    op0=ALU.mult, op1=ALU.add,
                )
                nc.gpsimd.tensor_mul(cv, cv, bt)
                nc.sync.dma_start_transpose(v_new[:, :, ec * P : (ec + 1) * P], cv)
            # prefetch wout + w1a into the slots that free at P1v's end
            wo_t = wts.tile([P, 2, EC, NQ], BF16, tag="s1")
            w1a_t = wts.tile([P, 4, KC, NQ], BF16, tag="s2")
            for n in range(2):
                nc.scalar.dma_start(wo_t[:, n], wo_d[:, n])
            for n in range(4):
                nc.scalar.dma_start(w1a_t[:, n], w1a_d[:, n])

            if stop_after == "v_new":
                dump3(v_new[:, :, 0:1024])
                return nc

            # ---------------- P6: attn = A @ v_new, fused LN2 stats ----------
            z2T = acts.tile([P, EC, L], BF16, tag="big1")
            for lc in range(LC):
                atn = acts.tile([P, E], BF16, tag="atn", bufs=2)
                ss = work.tile([P, 4], F32, tag="st4", bufs=4)
                sq = work.tile([P, 4], F32, tag="st4", bufs=4)
                for f in range(4):
                    ps = psum.tile([P, NQ], F32, tag="ps")
                    for lpc in range(LC):
                        nc.tensor.matmul(
                            ps,
                            AT[:, lpc, lc * P : (lc + 1) * P],
                            v_new[:, lpc, f * NQ : (f + 1) * NQ],
                            start=(lpc == 0),
                            stop=(lpc == LC - 1),
                        )
                    if attn_scale == 1.0:
                        nc.scalar.activation(
                            atn[:, f * NQ : (f + 1) * NQ], ps, AF.Copy,
                            accum_out=ss[:, f : f + 1],
                        )
                    else:
                        nc.scalar.activation(
                            atn[:, f * NQ : (f + 1) * NQ], ps, AF.Copy,
                            scale=float(attn_scale), accum_out=ss[:, f : f + 1],
                        )
                    nc.scalar.activation(
                        dead1.broadcast_to((P, NQ)), ps, AF.Square,
                        accum_out=sq[:, f : f + 1],
                    )
                s1t = work.tile([P, 1], F32, tag="st1", bufs=8)
                nc.vector.reduce_sum(s1t, ss, axis=AX.X)
                q1t = work.tile([P, 1], F32, tag="st1", bufs=8)
                nc.vector.reduce_sum(q1t, sq, axis=AX.X)
                mean, rstd = rstd_from_sums(s1t, q1t, E, scl=float(attn_scale))
                nc.vector.tensor_scalar(
                    atn, atn, mean, rstd, op0=ALU.subtract, op1=ALU.mult
                )
                nc.sync.dma_start_transpose(z2T[:, :, lc * P : (lc + 1) * P], atn)

            if stop_after == "z2T":
                dump3(z2T[:, 0:8, :])
                return nc

            # ---------------- P8: proj_out + residual -> xnew (SBUF, bf16) ---
            # LN1(xnew) stats accumulate in the evacuations; h2T transposes
            # happen per-lc right here.
            xnew = acts.tile([P, LC, H], BF16, tag="big3")
            h2Ta = acts.tile([P, KC, NQ], BF16, tag="hta")
            h2Tb = acts.tile([P, KC, NQ], BF16, tag="htb")
            for lc in range(LC):
                xt = work.tile([P, H], BF16, tag="xy")
                nc.sync.dma_start(xt, x_d[:, lc, :])
                xs = work.tile([P, 2], F32, tag="st4", bufs=4)
                xq = work.tile([P, 2], F32, tag="st4", bufs=4)
                for hc in range(2):
                    ps = psum.tile([P, NQ], F32, tag="ps")
                    for ec in range(EC):
                        nc.tensor.matmul(
                            ps,
                            z2T[:, ec, lc * P : (lc + 1) * P],
                            wo_t[:, hc, ec, :],
                            start=(ec == 0),
                            stop=(ec == EC - 1),
                        )
                    xsl = xnew[:, lc, hc * NQ : (hc + 1) * NQ]
                    nc.vector.scalar_tensor_tensor(
                        xsl, ps, 1.0, xt[:, hc * NQ : (hc + 1) * NQ],
                        op0=ALU.bypass, op1=ALU.add, accum_out=xs[:, hc : hc + 1],
                    )
                    nc.vector.scalar_tensor_tensor(
                        dead2.broadcast_to((P, NQ)), xsl, 1.0, xsl,
                        op0=ALU.bypass, op1=ALU.mult, accum_out=xq[:, hc : hc + 1],
                    )
                s1t = work.tile([P, 1], F32, tag="st1", bufs=8)
                nc.vector.reduce_sum(s1t, xs[:, 0:2], axis=AX.X)
                q1t = work.tile([P, 1], F32, tag="st1", bufs=8)
                nc.vector.reduce_sum(q1t, xq[:, 0:2], axis=AX.X)
                mean, rstd = rstd_from_sums(s1t, q1t, H)
                h2c = work.tile([P, H], BF16, tag="row2k")
                nc.vector.tensor_scalar(
                    h2c, xnew[:, lc, :], mean, rstd,
                    op0=ALU.subtract, op1=ALU.mult,
                )
                hdst = h2Ta if lc < 4 else h2Tb
                nc.sync.dma_start_transpose(
                    hdst[:, :, (lc % 4) * P : (lc % 4 + 1) * P], h2c
                )
            # w1b into the slot wout frees at P8's end
            w1b_t = wts.tile([P, 4, KC, NQ], BF16, tag="s1")
            for n in range(4):
                nc.scalar.dma_start(w1b_t[:, n], w1b_d[:, n])

            if stop_after == "h2T":
                dump3(h2Ta)
                return nc

            # ---------------- P10: mlp1 (gelu) -> ug_a, ug_b -----------------
            ug_a = acts.tile([P, JC // 2, L], BF16, tag="big2")
            ug_b = acts.tile([P, JC // 2, L], BF16, tag="big1")
            w2_t = [None, None]
            for half, (w1u, ugx) in enumerate(((w1a_t, ug_a), (w1b_t, ug_b))):
                for hf in range(2):
                    for jx in range(JC // 2):
                        ps = psum.tile([P, NQ], F32, tag="ps")
                        for kc in range(KC):
                            nc.tensor.matmul(
                                ps,
                                w1u[:, jx // 4, kc, (jx % 4) * P : (jx % 4 + 1) * P],
                                (h2Ta if hf == 0 else h2Tb)[:, kc, :],
                                start=(kc == 0),
                                stop=(kc == KC - 1),
                            )
                        nc.scalar.activation(
                            ugx[:, jx, hf * NQ : (hf + 1) * NQ], ps, AF.Gelu
                        )
                # prefetch w2 into the slot this half's w1 frees
                tag = "s2" if half == 0 else "s1"
                w2d = w2a_d if half == 0 else w2b_d
                w2_t[half] = wts.tile(
                    [P, 2, 16, NQ], BF16, tag=tag, name=f"w2_{half}"
                )
                for n in range(2):
                    nc.scalar.dma_start(w2_t[half][:, n], w2d[:, n])

            if stop_after == "ugT":
                dump3(ug_a[:, 0:8, :])
                return nc

            # ---------------- P11: mlp2 + residual -> y ----------------------
            for hc in range(2):
                w2u = w2_t[hc]
                for lc in range(LC):
                    ps = psum.tile([P, NQ], F32, tag="ps")
                    for jc in range(JC):
                        ugx = ug_a if jc < JC // 2 else ug_b
                        nc.tensor.matmul(
                            ps,
                            ugx[:, jc % (JC // 2), lc * P : (lc + 1) * P],
                            w2u[:, jc // 16, jc % 16, :],
                            start=(jc == 0),
                            stop=(jc == JC - 1),
                        )
                    yh = work.tile([P, NQ], F32, tag="yh", bufs=1)
                    nc.vector.tensor_add(
                        yh, ps, xnew[:, lc, hc * NQ : (hc + 1) * NQ]
                    )
                    nc.sync.dma_start(y_d[:, lc, hc * NQ : (hc + 1) * NQ], yh)
    return nc


def _legalize_waits(nc, limit=1):
    """Split excess sync waits onto same-engine NOPs (walrus rejects >limit)."""
    cnt = 0
    for fn in nc.m.functions:
        for bb in fn.blocks:
            insts = bb.instructions
            fixes = []
            for idx, ins in enumerate(insts):
                si = ins.sync_info
                if si is None or not si.on_wait or len(si.on_wait) <= limit:
                    continue
                waits = list(si.on_wait)
                excess, keep = waits[:-limit], waits[-limit:]
                nops = []
                for j in range(0, len(excess), limit):
                    nop = mybir.InstNoOp(name=f"WFIX-{cnt}", text_hint="waitfix")
                    cnt += 1
                    nop.engine = ins.engine
                    nop.sync_info = mybir.SyncInfo(
                        on_wait=excess[j : j + limit], on_update=[]
                    )
                    nops.append(nop)
                si.on_wait = keep
                fixes.append((idx, nops))
            for idx, nops in reversed(fixes):
                for nop in reversed(nops):
                    insts.insert(idx, nop)
    return cnt


def _to_pchunk(a2d, nchunk):
    """[R, C] with R = nchunk*128 -> [128, nchunk, C] (p-major layout)."""
    R, C = a2d.shape
    return np.ascontiguousarray(a2d.reshape(nchunk, P, C).transpose(1, 0, 2))


def _col_chunks(a, nn):
    """[P, KCx, C] -> [P, nn, KCx, C//nn] column-chunk-major."""
    Pp, kk, C = a.shape
    w = C // nn
    return np.ascontiguousarray(
        np.stack([a[:, :, n * w : (n + 1) * w] for n in range(nn)], axis=1)
    )


def _prep_inputs(inputs):
    f32 = lambda a: np.asarray(a, np.float32)
    bf = lambda a: np.ascontiguousarray(a.astype(ml_dtypes.bfloat16))

    x = f32(inputs["x"])
    ln1_w, ln1_b = f32(inputs["ln1_w"]), f32(inputs["ln1_b"])
    ln2_w, ln2_b = f32(inputs["ln2_w"]), f32(inputs["ln2_b"])
    w_qkv, b_qkv = f32(inputs["w_qkv"]), f32(inputs["b_qkv"])
    w_out, b_out = f32(inputs["w_out"]), f32(inputs["b_out"])
    rel_pos = f32(inputs["rel_pos"])
    w_beta, b_beta = f32(inputs["w_beta"]), f32(inputs["b_beta"])
    w1, b1 = f32(inputs["w1"]), f32(inputs["b1"])
    w2, b2 = f32(inputs["w2"]), f32(inputs["b2"])
    conv_w = f32(inputs["conv_w"])
    attn_scale = float(np.asarray(inputs["attn_scale"]).reshape(-1)[0])

    assert not np.any(b_qkv), "nonzero qkv bias not supported"
    assert not np.any(b_out) and not np.any(b2), "nonzero row bias not supported"

    # fold LN affine into the consuming matmuls
    wqkv_e = w_qkv * ln1_w[None, :]
    bqkv_e = b_qkv + w_qkv @ ln1_b
    assert np.allclose(bqkv_e, 0.0), "nonzero folded qkv bias not supported"
    wq_e, wk_e, wv_e = wqkv_e[:E], wqkv_e[E : 2 * E], wqkv_e[2 * E :]

    # beta: comb=[h, pos_info] trick -> rank-1 update, then LN fold
    p_bar = rel_pos[:L].mean(0)
    s = w_beta[:, H:].sum(1)
    wb_raw = w_beta[:, :H] + np.outer(s, p_bar)
    wb_e = wb_raw * ln1_w[None, :]
    bb_e = b_beta + wb_raw @ ln1_b
    assert np.allclose(bb_e, 0.0), "nonzero folded beta bias not supported"

    wout_e = w_out * ln2_w[None, :]
    bout_e = b_out + w_out @ ln2_b
    assert np.allclose(bout_e, 0.0), "nonzero folded out bias not supported"

    w1_e = w1 * ln1_w[None, :]
    b1_e = b1 + w1 @ ln1_b
    assert np.allclose(b1_e, 0.0), "nonzero folded mlp1 bias not supported"

    # conv diag blocks: cd[p, ec, t, m] = conv_w[ec*128+p, 0, t] if p==m else 0
    cd = np.zeros((P, EC, 3, P), np.float32)
    idx = np.arange(P)
    cd[idx, :, :, idx] = conv_w[:, 0, :].reshape(EC, P, 3).transpose(1, 0, 2)
    # vector-conv weights for v: cwv[p, t, ec] = conv_w[ec*128+p, 0, t]
    cwv = np.ascontiguousarray(
        conv_w[:, 0, :].reshape(EC, P, 3).transpose(1, 2, 0)
    )

    w2T = _to_pchunk(w2.T, JC)  # [P, JC, H]

    shared = {
        "wq": bf(_col_chunks(_to_pchunk(wq_e.T, KC), 4)),
        "wk": bf(_col_chunks(_to_pchunk(wk_e.T, KC), 4)),
        "wv": bf(_col_chunks(_to_pchunk(wv_e.T, KC), 4)),
        "wb": bf(_col_chunks(_to_pchunk(wb_e.T, KC), 4)),
        "wo": bf(_col_chunks(_to_pchunk(wout_e.T, EC), 2)),
        "w1a": bf(_col_chunks(_to_pchunk(w1_e.T, KC)[:, :, :E], 4)),
        "w1b": bf(_col_chunks(_to_pchunk(w1_e.T, KC)[:, :, E:], 4)),
        "w2a": bf(np.ascontiguousarray(w2T[:, :, :NQ].reshape(P, 2, 16, NQ))),
        "w2b": bf(np.ascontiguousarray(w2T[:, :, NQ:].reshape(P, 2, 16, NQ))),
        "cdiag": bf(cd),
        "cwv": np.ascontiguousarray(cwv, dtype=np.float32),
    }
    in_maps = []
    for b in range(B):
        m = dict(shared)
        m["x"] = bf(x[b].reshape(LC, P, H).transpose(1, 0, 2))
        in_maps.append(m)
    return in_maps, attn_scale


def kernel(**inputs) -> np.ndarray:
    in_maps, attn_scale = _prep_inputs(inputs)
    nc = _build_program(attn_scale)
    _legalize_waits(nc)
    res = run_bass_kernel_spmd(nc, in_maps, core_ids=list(range(B)), trace=TRACE)
    LAST["exec_time_ns"] = res.exec_time_ns
    LAST["results"] = res
    out = np.empty((B, L, H), np.float32)
    for b in range(B):
        yb = np.asarray(res.results[b]["y"])  # [128, LC, H]
        out[b] = yb.transpose(1, 0, 2).reshape(L, H)
    return out


# revision 11
# speedup vs baseline: 1.3633x; 1.0166x over previous
"""DeltaNet block kernel for Trainium2, data-parallel over batch (8 cores).

v3 strategy (per core, one batch element; L=1024, H=1024, E=2048):
  - LN affine params and the pos_embed rank-1 term folded into effective
    weights on the host (exact algebra); all row biases are exactly zero for
    this problem (asserted).
  - Delta-rule einsums in attention form: out = (q @ k^T) @ (beta*v).
  - All matmuls bf16 with fp32 PSUM accumulation; x is carried in bf16.
  - Activation-table discipline: scalar engine sequence is
    {Sqrt} -> {Silu,Square} -> {Sigmoid,Erf,Copy} -> {Copy,Sqrt} -> {Gelu};
    copy/square live in every table set, so ~5 table loads total.  gelu for
    v is computed via erf (sigmoid set): g = x*(erf(x/sqrt2)+1) = 2*gelu(x),
    with the 1/2 folded into the beta gate (beta' = 0.45*sig + 0.05).
  - q/k row-norm rsqrt via bit-trick + 2 Newton steps on the vector engine,
    both rows packed in one [P,2] chain.
  - Depthwise conv(k=3): q/k as 3 accumulating diagonal matmuls on the PE
    (diag blocks DMA'd per-chunk on the gpsimd SWDGE queue); v as 3 fused
    scalar_tensor_tensor taps on the vector engine over a zero-guarded row,
    with the beta gate and final multiply on gpsimd.
  - LN2 / pre-MLP LN1 statistics accumulate during psum evacuations
    (activation accum_out / stt accum_out), collapsing the LN phases.
  - x + attn_out residual (xnew) stays in SBUF in bf16.
  - Weight DMAs are column-chunked and rotate through two 32KB SBUF slots
    (s1: wq->wv->wout->w1b->w2b, s2: wk->wb->w1a->w2a); triggers are emitted
    right after the previous tenant's last compute use so transfers hide
    under compute.
  - hT and h2T are split into column-half tiles so consumers only wait on
    the transposes they actually need (xbar-transpose writes are tracked
    coarsely per tile).
"""

import sys

import numpy as np

sys.path.insert(0, "/opt/trn_rl_repo")

import ml_dtypes  # noqa: E402

import concourse.bass as bass  # noqa: E402
import concourse.mybir as mybir  # noqa: E402
import concourse.tile as tile  # noqa: E402
from concourse.bass_utils import run_bass_kernel_spmd  # noqa: E402

BF16 = mybir.dt.bfloat16
F32 = mybir.dt.float32
I32 = mybir.dt.int32
AF = mybir.ActivationFunctionType
ALU = mybir.AluOpType
AX = mybir.AxisListType

B, L, H, E = 8, 1024, 1024, 2048
P = 128
LC = L // P    # 8
KC = H // P    # 8
EC = E // P    # 16
JC = 4 * H // P  # 32
NQ = 512
EPS = 1e-5
RSQRT_MAGIC = 0x5F3759DF
INV_SQRT2 = 0.7071067811865476

TRACE = False
LAST = {}


def _build_program(attn_scale: float, stop_after: str | None = None):
    nc = bass.Bass("TRN2", target_bir_lowering=False)

    x_d = nc.dram_tensor("x", [P, LC, H], BF16, kind="ExternalInput")
    wq_d = nc.dram_tensor("wq", [P, 4, KC, NQ], BF16, kind="ExternalInput")
    wk_d = nc.dram_tensor("wk", [P, 4, KC, NQ], BF16, kind="ExternalInput")
    wv_d = nc.dram_tensor("wv", [P, 4, KC, NQ], BF16, kind="ExternalInput")
    wb_d = nc.dram_tensor("wb", [P, 4, KC, NQ], BF16, kind="ExternalInput")
    wo_d = nc.dram_tensor("wo", [P, 2, EC, NQ], BF16, kind="ExternalInput")
    w1a_d = nc.dram_tensor("w1a", [P, 4, KC, NQ], BF16, kind="ExternalInput")
    w1b_d = nc.dram_tensor("w1b", [P, 4, KC, NQ], BF16, kind="ExternalInput")
    w2a_d = nc.dram_tensor("w2a", [P, 2, 16, NQ], BF16, kind="ExternalInput")
    w2b_d = nc.dram_tensor("w2b", [P, 2, 16, NQ], BF16, kind="ExternalInput")
    cdiag_d = nc.dram_tensor("cdiag", [P, EC, 3, P], BF16, kind="ExternalInput")
    cwv_d = nc.dram_tensor("cwv", [P, 3, EC], F32, kind="ExternalInput")
    y_d = nc.dram_tensor("y", [P, LC, H], F32, kind="ExternalOutput")

    with tile.TileContext(nc) as tc:
        with (
            tc.tile_pool(name="consts", bufs=1) as consts,
            tc.tile_pool(name="wts", bufs=1) as wts,
            tc.tile_pool(name="acts", bufs=1) as acts,
            tc.tile_pool(name="work", bufs=2) as work,
            tc.tile_pool(name="psum", bufs=8, space="PSUM") as psum,
        ):
            zero_t = consts.tile([P, 1], F32)
            nc.vector.memset(zero_t, 0.0)
            nc.const_aps.aps[(F32, 0.0)] = zero_t[:]
            eps_t = consts.tile([P, 1], F32)
            nc.vector.memset(eps_t, EPS)
            c15_t = consts.tile([P, 2], F32)
            nc.vector.memset(c15_t, 1.5)
            cwv = consts.tile([P, 3, EC], F32)
            nc.sync.dma_start(cwv, cwv_d[:])
            dead1 = consts.tile([P, 1], BF16)  # scalar-engine dead store
            dead2 = consts.tile([P, 1], BF16)  # vector-engine dead store

            def rsqrt2_vec(ssq2):
                """[P,2] f32 sums-of-squares -> [P,2] f32 rsqrt (bit trick +
                2 Newton steps), both lanes in one chain."""
                se = work.tile([P, 2], F32, tag="nt", bufs=8)
                nc.vector.tensor_scalar_add(se, ssq2, 1e-20)
                hh = work.tile([P, 2], F32, tag="nt", bufs=8)
                nc.vector.tensor_scalar_mul(hh, se, -0.5)
                r = work.tile([P, 2], F32, tag="nt", bufs=8)
                nc.vector.tensor_scalar(
                    r.bitcast(I32), se.bitcast(I32), 1, -1,
                    op0=ALU.arith_shift_right, op1=ALU.bitwise_xor,
                )
                nc.vector.tensor_scalar_add(
                    r.bitcast(I32), r.bitcast(I32), RSQRT_MAGIC + 1
                )
                for _ in range(2):
                    yy = work.tile([P, 2], F32, tag="nt", bufs=8)
                    nc.vector.tensor_mul(yy, r, r)
                    ww = work.tile([P, 2], F32, tag="nt", bufs=8)
                    nc.vector.tensor_mul(ww, yy, hh)
                    nc.vector.tensor_add(ww, ww, c15_t)
                    r2 = work.tile([P, 2], F32, tag="nt", bufs=8)
                    nc.vector.tensor_mul(r2, ww, r)
                    r = r2
                return r

            def rstd_from_sums(ssum, sqsum, n, scl=1.0):
                """[P,1] sums of x (scaled by scl) and x^2 (raw) -> mean, rstd."""
                mean = work.tile([P, 1], F32, tag="st1", bufs=8)
                nc.vector.tensor_scalar_mul(mean, ssum, 1.0 / n)
                ex2 = work.tile([P, 1], F32, tag="st1", bufs=8)
                nc.vector.tensor_scalar_mul(ex2, sqsum, scl * scl / n)
                var = work.tile([P, 1], F32, tag="st1", bufs=8)
                nc.vector.tensor_mul(var, mean, mean)
                nc.vector.tensor_sub(var, ex2, var)
                r = work.tile([P, 1], F32, tag="st1", bufs=8)
                nc.scalar.activation(r, var, AF.Sqrt, bias=eps_t[:])
                nc.vector.reciprocal(r, r)
                return mean, r

            def conv3(ps, row, hf, dg):
                """3-tap PE conv into psum ps [P,NQ]; row [P,L] one e-chunk."""
                base = hf * NQ
                nc.tensor.matmul(
                    ps, dg[:, 1, :], row[:, base : base + NQ],
                    start=True, stop=False,
                )
                if hf == 0:
                    nc.tensor.matmul(
                        ps[:, 1:NQ], dg[:, 0, :], row[:, 0 : NQ - 1],
                        start=False, stop=False, skip_group_check=True,
                    )
                    nc.tensor.matmul(
                        ps, dg[:, 2, :], row[:, 1 : NQ + 1],
                        start=False, stop=True, skip_group_check=True,
                    )
                else:
                    nc.tensor.matmul(
                        ps[:, 0 : NQ - 1], dg[:, 2, :], row[:, base + 1 : L],
                        start=False, stop=False, skip_group_check=True,
                    )
                    nc.tensor.matmul(
                        ps, dg[:, 0, :], row[:, base - 1 : base - 1 + NQ],
                        start=False, stop=True, skip_group_check=True,
                    )

            def dump3(src_ap):
                wdt = src_ap.shape[-1]
                for c in range(src_ap.shape[1]):
                    tmp = work.tile([P, H], F32, tag="dbg")
                    nc.vector.tensor_copy(tmp[:, 0:wdt], src_ap[:, c, :])
                    nc.sync.dma_start(y_d[:, c, 0:wdt], tmp[:, 0:wdt])

            # ---------------- P0: LN1(x) -> hTa/hTb [P, KC, NQ] bf16 ---------
            hTa = acts.tile([P, KC, NQ], BF16, tag="hta")
            hTb = acts.tile([P, KC, NQ], BF16, tag="htb")
            for lc in range(LC):
                xt = work.tile([P, H], BF16, tag="row2k", bufs=3)
                nc.sync.dma_start(xt, x_d[:, lc, :])
                bnst = work.tile([P, 2, 6], F32, tag="bnst")
                x3 = xt.rearrange("p (s f) -> p s f", s=2)
                for s in range(2):
                    nc.vector.bn_stats(bnst[:, s, :], x3[:, s, :])
                mv = work.tile([P, 2], F32, tag="mv")
                nc.vector.bn_aggr(mv, bnst)
                rstd = work.tile([P, 1], F32, tag="st1", bufs=8)
                nc.scalar.activation(rstd, mv[:, 1:2], AF.Sqrt, bias=eps_t[:])
                nc.vector.reciprocal(rstd, rstd)
                z = work.tile([P, H], BF16, tag="row2k", bufs=3)
                nc.vector.tensor_scalar(
                    z, xt, mv[:, 0:1], rstd, op0=ALU.subtract, op1=ALU.mult
                )
                hdst = hTa if lc < 4 else hTb
                nc.sync.dma_start_transpose(
                    hdst[:, :, (lc % 4) * P : (lc % 4 + 1) * P], z
                )

            def hT_st(kc, lc):
                """stationary hT block [P,P] for token block lc."""
                src = hTa if lc < 4 else hTb
                return src[:, kc, (lc % 4) * P : (lc % 4 + 1) * P]

            if stop_after == "h":
                dump3(hTa)
                return nc

            # ---------------- P3: q,k + silu + normalize-mix -> qT,kT --------
            qT = acts.tile([P, EC, L], BF16, tag="big1")
            kT = acts.tile([P, EC, L], BF16, tag="big2")
            wq_t = wts.tile([P, 4, KC, NQ], BF16, tag="s1")
            wk_t = wts.tile([P, 4, KC, NQ], BF16, tag="s2")
            for n in range(4):
                nc.scalar.dma_start(wq_t[:, n], wq_d[:, n])
            for n in range(4):
                nc.scalar.dma_start(wk_t[:, n], wk_d[:, n])
            for lc in range(LC):
                qs = work.tile([P, E], BF16, tag="qk", bufs=5)
                ks = work.tile([P, E], BF16, tag="qk", bufs=5)
                sq4 = work.tile([P, 4], F32, tag="st4", bufs=4)
                sk4 = work.tile([P, 4], F32, tag="st4", bufs=4)
                for wu, dst, acc4 in ((wq_t, qs, sq4), (wk_t, ks, sk4)):
                    for n in range(4):
                        ps = psum.tile([P, NQ], F32, tag="ps")
                        for kc in range(KC):
                            nc.tensor.matmul(
                                ps,
                                hT_st(kc, lc),
                                wu[:, n, kc, :],
                                start=(kc == 0),
                                stop=(kc == KC - 1),
                            )
                        chunk = dst[:, n * NQ : (n + 1) * NQ]
                        nc.scalar.activation(chunk, ps, AF.Silu)
                        nc.vector.scalar_tensor_tensor(
                            dead2.broadcast_to((P, NQ)), chunk, 1.0, chunk,
                            op0=ALU.bypass, op1=ALU.mult,
                            accum_out=acc4[:, n : n + 1],
                        )
                ssq2 = work.tile([P, 2], F32, tag="st4", bufs=4)
                nc.vector.reduce_sum(ssq2[:, 0:1], sq4, axis=AX.X)
                nc.vector.reduce_sum(ssq2[:, 1:2], sk4, axis=AX.X)
                r2 = rsqrt2_vec(ssq2)
                s = work.tile([P, E], BF16, tag="qk", bufs=5)
                nc.vector.tensor_scalar_mul(s, ks, 0.1)
                nc.vector.tensor_scalar_mul(qs, qs, r2[:, 0:1])
                nc.vector.tensor_add(qs, qs, s)
                nc.sync.dma_start_transpose(qT[:, :, lc * P : (lc + 1) * P], qs)
                nc.vector.tensor_scalar_mul(s, qs, 0.1)
                nc.vector.tensor_scalar_mul(ks, ks, r2[:, 1:2])
                nc.vector.tensor_add(ks, ks, s)
                nc.sync.dma_start_transpose(kT[:, :, lc * P : (lc + 1) * P], ks)

            if stop_after == "qT":
                dump3(qT[:, 0:8, :])
                return nc

            # ---------------- P4: conv q,k in place (diag matmuls) -----------
            # wv/wb triggers first: WAR on wq/wk (free at P3's last matmul),
            # so they fire immediately and stream during P4+P5.
            wv_t = wts.tile([P, 4, KC, NQ], BF16, tag="s1")
            wb_t = wts.tile([P, 4, KC, NQ], BF16, tag="s2")
            for n in range(4):
                nc.scalar.dma_start(wv_t[:, n], wv_d[:, n])
            for n in range(4):
                nc.scalar.dma_start(wb_t[:, n], wb_d[:, n])
            # conv diag blocks stream on the gpsimd SWDGE queue (depth-2 ahead)
            cdgs = {}
            for e2 in range(2):
                cdg = work.tile([P, 3, P], BF16, tag="cdg", bufs=2, name=f"cdg{e2}")
                nc.gpsimd.dma_start(cdg, cdiag_d[:, e2])
                cdgs[e2] = cdg
            for ec in range(EC):
                if ec + 2 < EC:
                    cdg = work.tile(
                        [P, 3, P], BF16, tag="cdg", bufs=2, name=f"cdg{ec + 2}"
                    )
                    nc.gpsimd.dma_start(cdg, cdiag_d[:, ec + 2])
                    cdgs[ec + 2] = cdg
                dg = cdgs.pop(ec)
                for tz in (qT, kT):
                    ps0 = psum.tile([P, NQ], F32, tag="ps")
                    conv3(ps0, tz[:, ec, :], 0, dg)
                    ps1 = psum.tile([P, NQ], F32, tag="ps")
                    conv3(ps1, tz[:, ec, :], 1, dg)
                    nc.scalar.copy(tz[:, ec, 0:NQ], ps0)
                    nc.scalar.copy(tz[:, ec, NQ : 2 * NQ], ps1)

            if stop_after == "qTc":
                dump3(qT[:, 0:8, :])
                return nc

            # ---------------- P5: AT = (q @ k^T)^T chunks --------------------
            AT = acts.tile([P, LC, L], BF16, tag="big3")
            for lpc in range(LC):
                for hf in range(2):
                    ps = psum.tile([P, NQ], F32, tag="ps")
                    for ec in range(EC):
                        nc.tensor.matmul(
                            ps,
                            kT[:, ec, lpc * P : (lpc + 1) * P],
                            qT[:, ec, hf * NQ : (hf + 1) * NQ],
                            start=(ec == 0),
                            stop=(ec == EC - 1),
                        )
                    nc.scalar.copy(AT[:, lpc, hf * NQ : (hf + 1) * NQ], ps)

            if stop_after == "AT":
                dump3(AT)
                return nc

            # ---------------- P1v: v,beta + erf-gelu + vec conv + transpose --
            v_new = acts.tile([P, LC, E], BF16, tag="big2")
            for ec in range(EC):
                vt = work.tile([P, L + 2], BF16, tag="vt")
                nc.vector.memset(vt[:, 0:1], 0.0)
                nc.vector.memset(vt[:, L + 1 : L + 2], 0.0)
                bt = work.tile([P, L], BF16, tag="row2k", bufs=3)
                for hf in range(2):
                    ps = psum.tile([P, NQ], F32, tag="ps")
                    for kc in range(KC):
                        nc.tensor.matmul(
                            ps,
                            wb_t[:, ec // 4, kc, (ec % 4) * P : (ec % 4 + 1) * P],
                            (hTa if hf == 0 else hTb)[:, kc, :],
                            start=(kc == 0),
                            stop=(kc == KC - 1),
                        )
                    nc.scalar.activation(
                        bt[:, hf * NQ : (hf + 1) * NQ], ps, AF.Sigmoid
                    )
                # beta' = (0.9*sig + 0.1)/2 ; the 1/2 undoes g = 2*gelu(v)
                nc.vector.tensor_scalar(bt, bt, 0.45, 0.05, op0=ALU.mult, op1=ALU.add)
                for hf in range(2):
                    ps = psum.tile([P, NQ], F32, tag="ps")
                    for kc in range(KC):
                        nc.tensor.matmul(
                            ps,
                            wv_t[:, ec // 4, kc, (ec % 4) * P : (ec % 4 + 1) * P],
                            (hTa if hf == 0 else hTb)[:, kc, :],
                            start=(kc == 0),
                            stop=(kc == KC - 1),
                        )
                    et = work.tile([P, NQ], BF16, tag="et", bufs=1)
                    nc.scalar.activation(et, ps, AF.Erf, scale=INV_SQRT2)
                    # et+1 on scalar (Copy table): g = ps*(erf+1) = 2*gelu(ps)
                    nc.scalar.activation(et, et, AF.Copy, bias=1.0)
                    nc.vector.tensor_mul(
                        vt[:, 1 + hf * NQ : 1 + (hf + 1) * NQ], ps, et
                    )
                cv = work.tile([P, L], BF16, tag="cv", bufs=2)
                nc.vector.tensor_scalar_mul(cv, vt[:, 0:L], cwv[:, 0, ec : ec + 1])
                nc.vector.scalar_tensor_tensor(
                    cv, vt[:, 1 : L + 1], cwv[:, 1, ec : ec + 1], cv,
                    op0=ALU.mult, op1=ALU.add,
                )
                nc.vector.scalar_tensor_tensor(
                    cv, vt[:, 2 : L + 2], cwv[:, 2, ec : ec + 1], cv,
                    op0=ALU.mult, op1=ALU.add,
                )
                nc.vector.tensor_mul(cv, cv, bt)
                nc.sync.dma_start_transpose(v_new[:, :, ec * P : (ec + 1) * P], cv)
            # prefetch wout + w1a into the slots that free at P1v's end
            wo_t = wts.tile([P, 2, EC, NQ], BF16, tag="s1")
            w1a_t = wts.tile([P, 4, KC, NQ], BF16, tag="s2")
            for n in range(2):
                nc.scalar.dma_start(wo_t[:, n], wo_d[:, n])
            for n in range(4):
                nc.scalar.dma_start(w1a_t[:, n], w1a_d[:, n])

            if stop_after == "v_new":
                dump3(v_new[:, :, 0:1024])
                return nc

            # ---------------- P6: attn = A @ v_new, fused LN2 stats ----------
            z2T = acts.tile([P, EC, L], BF16, tag="big1")
            for lc in range(LC):
                atn = acts.tile([P, E], BF16, tag="atn", bufs=2)
                ss = work.tile([P, 4], F32, tag="st4", bufs=4)
                sq = work.tile([P, 4], F32, tag="st4", bufs=4)
                for f in range(4):
                    ps = psum.tile([P, NQ], F32, tag="ps")
                    for lpc in range(LC):
                        nc.tensor.matmul(
                            ps,
                            AT[:, lpc, lc * P : (lc + 1) * P],
                            v_new[:, lpc, f * NQ : (f + 1) * NQ],
                            start=(lpc == 0),
                            stop=(lpc == LC - 1),
                        )
                    if attn_scale == 1.0:
                        nc.scalar.activation(
                            atn[:, f * NQ : (f + 1) * NQ], ps, AF.Copy,
                            accum_out=ss[:, f : f + 1],
                        )
                    else:
                        nc.scalar.activation(
                            atn[:, f * NQ : (f + 1) * NQ], ps, AF.Copy,
                            scale=float(attn_scale), accum_out=ss[:, f : f + 1],
                        )
                    nc.scalar.activation(
                        dead1.broadcast_to((P, NQ)), ps, AF.Square,
                        accum_out=sq[:, f : f + 1],
                    )
                s1t = work.tile([P, 1], F32, tag="st1", bufs=8)
                nc.vector.reduce_sum(s1t, ss, axis=AX.X)
                q1t = work.tile([P, 1], F32, tag="st1", bufs=8)
                nc.vector.reduce_sum(q1t, sq, axis=AX.X)
                mean, rstd = rstd_from_sums(s1t, q1t, E, scl=float(attn_scale))
                nc.vector.tensor_scalar(
                    atn, atn, mean, rstd, op0=ALU.subtract, op1=ALU.mult
                )
                nc.sync.dma_start_transpose(z2T[:, :, lc * P : (lc + 1) * P], atn)

            if stop_after == "z2T":
                dump3(z2T[:, 0:8, :])
                return nc

            # ---------------- P8: proj_out + residual -> xnew (SBUF, bf16) ---
            # LN1(xnew) stats accumulate in the evacuations; h2T transposes
            # happen per-lc right here.
            xnew = acts.tile([P, LC, H], BF16, tag="big3")
            h2Ta = acts.tile([P, KC, NQ], BF16, tag="hta")
            h2Tb = acts.tile([P, KC, NQ], BF16, tag="htb")
            for lc in range(LC):
                xt = work.tile([P, H], BF16, tag="row2k", bufs=3)
                nc.sync.dma_start(xt, x_d[:, lc, :])
                xs = work.tile([P, 2], F32, tag="st4", bufs=4)
                xq = work.tile([P, 2], F32, tag="st4", bufs=4)
                for hc in range(2):
                    ps = psum.tile([P, NQ], F32, tag="ps")
                    for ec in range(EC):
                        nc.tensor.matmul(
                            ps,
                            z2T[:, ec, lc * P : (lc + 1) * P],
                            wo_t[:, hc, ec, :],
                            start=(ec == 0),
                            stop=(ec == EC - 1),
                        )
                    xsl = xnew[:, lc, hc * NQ : (hc + 1) * NQ]
                    nc.vector.scalar_tensor_tensor(
                        xsl, ps, 1.0, xt[:, hc * NQ : (hc + 1) * NQ],
                        op0=ALU.bypass, op1=ALU.add, accum_out=xs[:, hc : hc + 1],
                    )
                    nc.vector.scalar_tensor_tensor(
                        dead2.broadcast_to((P, NQ)), xsl, 1.0, xsl,
                        op0=ALU.bypass, op1=ALU.mult, accum_out=xq[:, hc : hc + 1],
                    )
                s1t = work.tile([P, 1], F32, tag="st1", bufs=8)
                nc.vector.reduce_sum(s1t, xs[:, 0:2], axis=AX.X)
                q1t = work.tile([P, 1], F32, tag="st1", bufs=8)
                nc.vector.reduce_sum(q1t, xq[:, 0:2], axis=AX.X)
                mean, rstd = rstd_from_sums(s1t, q1t, H)
                h2c = work.tile([P, H], BF16, tag="row2k", bufs=3)
                nc.vector.tensor_scalar(
                    h2c, xnew[:, lc, :], mean, rstd,
                    op0=ALU.subtract, op1=ALU.mult,
                )
                hdst = h2Ta if lc < 4 else h2Tb
                nc.sync.dma_start_transpose(
                    hdst[:, :, (lc % 4) * P : (lc % 4 + 1) * P], h2c
                )
            # w1b into the slot wout frees at P8's end
            w1b_t = wts.tile([P, 4, KC, NQ], BF16, tag="s1")
            for n in range(4):
                nc.scalar.dma_start(w1b_t[:, n], w1b_d[:, n])

            if stop_after == "h2T":
                dump3(h2Ta)
                return nc

            # ---------------- P10: mlp1 (gelu) -> ug_a, ug_b -----------------
            ug_a = acts.tile([P, JC // 2, L], BF16, tag="big2")
            ug_b = acts.tile([P, JC // 2, L], BF16, tag="big1")
            w2_t = [None, None]
            for half, (w1u, ugx) in enumerate(((w1a_t, ug_a), (w1b_t, ug_b))):
                for hf in range(2):
                    for jx in range(JC // 2):
                        ps = psum.tile([P, NQ], F32, tag="ps")
                        for kc in range(KC):
                            nc.tensor.matmul(
                                ps,
                                w1u[:, jx // 4, kc, (jx % 4) * P : (jx % 4 + 1) * P],
                                (h2Ta if hf == 0 else h2Tb)[:, kc, :],
                                start=(kc == 0),
                                stop=(kc == KC - 1),
                            )
                        nc.scalar.activation(
                            ugx[:, jx, hf * NQ : (hf + 1) * NQ], ps, AF.Gelu
                        )
                # prefetch w2 into the slot this half's w1 frees
                tag = "s2" if half == 0 else "s1"
                w2d = w2a_d if half == 0 else w2b_d
                w2_t[half] = wts.tile(
                    [P, 2, 16, NQ], BF16, tag=tag, name=f"w2_{half}"
                )
                for n in range(2):
                    nc.scalar.dma_start(w2_t[half][:, n], w2d[:, n])

            if stop_after == "ugT":
                dump3(ug_a[:, 0:8, :])
                return nc

            # ---------------- P11: mlp2 + residual -> y ----------------------
            for hc in range(2):
                w2u = w2_t[hc]
                for lc in range(LC):
                    ps = psum.tile([P, NQ], F32, tag="ps")
                    for jc in range(JC):
                        ugx = ug_a if jc < JC // 2 else ug_b
                        nc.tensor.matmul(
                            ps,
                            ugx[:, jc % (JC // 2), lc * P : (lc + 1) * P],
                            w2u[:, jc // 16, jc % 16, :],
                            start=(jc == 0),
                            stop=(jc == JC - 1),
                        )
                    yh = work.tile([P, NQ], F32, tag="yh", bufs=1)
                    nc.vector.tensor_add(
                        yh, ps, xnew[:, lc, hc * NQ : (hc + 1) * NQ]
                    )
                    nc.sync.dma_start(y_d[:, lc, hc * NQ : (hc + 1) * NQ], yh)
    return nc


def _legalize_waits(nc, limit=1):
    """Split excess sync waits onto same-engine NOPs (walrus rejects >limit)."""
    cnt = 0
    for fn in nc.m.functions:
        for bb in fn.blocks:
            insts = bb.instructions
            fixes = []
            for idx, ins in enumerate(insts):
                si = ins.sync_info
                if si is None or not si.on_wait or len(si.on_wait) <= limit:
                    continue
                waits = list(si.on_wait)
                excess, keep = waits[:-limit], waits[-limit:]
                nops = []
                for j in range(0, len(excess), limit):
                    nop = mybir.InstNoOp(name=f"WFIX-{cnt}", text_hint="waitfix")
                    cnt += 1
                    nop.engine = ins.engine
                    nop.sync_info = mybir.SyncInfo(
                        on_wait=excess[j : j + limit], on_update=[]
                    )
                    nops.append(nop)
                si.on_wait = keep
                fixes.append((idx, nops))
            for idx, nops in reversed(fixes):
                for nop in reversed(nops):
                    insts.insert(idx, nop)
    return cnt


def _to_pchunk(a2d, nchunk):
    """[R, C] with R = nchunk*128 -> [128, nchunk, C] (p-major layout)."""
    R, C = a2d.shape
    return np.ascontiguousarray(a2d.reshape(nchunk, P, C).transpose(1, 0, 2))


def _col_chunks(a, nn):
    """[P, KCx, C] -> [P, nn, KCx, C//nn] column-chunk-major."""
    Pp, kk, C = a.shape
    w = C // nn
    return np.ascontiguousarray(
        np.stack([a[:, :, n * w : (n + 1) * w] for n in range(nn)], axis=1)
    )


def _prep_inputs(inputs):
    f32 = lambda a: np.asarray(a, np.float32)
    bf = lambda a: np.ascontiguousarray(a.astype(ml_dtypes.bfloat16))

    x = f32(inputs["x"])
    ln1_w, ln1_b = f32(inputs["ln1_w"]), f32(inputs["ln1_b"])
    ln2_w, ln2_b = f32(inputs["ln2_w"]), f32(inputs["ln2_b"])
    w_qkv, b_qkv = f32(inputs["w_qkv"]), f32(inputs["b_qkv"])
    w_out, b_out = f32(inputs["w_out"]), f32(inputs["b_out"])
    rel_pos = f32(inputs["rel_pos"])
    w_beta, b_beta = f32(inputs["w_beta"]), f32(inputs["b_beta"])
    w1, b1 = f32(inputs["w1"]), f32(inputs["b1"])
    w2, b2 = f32(inputs["w2"]), f32(inputs["b2"])
    conv_w = f32(inputs["conv_w"])
    attn_scale = float(np.asarray(inputs["attn_scale"]).reshape(-1)[0])

    assert not np.any(b_qkv), "nonzero qkv bias not supported"
    assert not np.any(b_out) and not np.any(b2), "nonzero row bias not supported"

    # fold LN affine into the consuming matmuls
    wqkv_e = w_qkv * ln1_w[None, :]
    bqkv_e = b_qkv + w_qkv @ ln1_b
    assert np.allclose(bqkv_e, 0.0), "nonzero folded qkv bias not supported"
    wq_e, wk_e, wv_e = wqkv_e[:E], wqkv_e[E : 2 * E], wqkv_e[2 * E :]

    # beta: comb=[h, pos_info] trick -> rank-1 update, then LN fold
    p_bar = rel_pos[:L].mean(0)
    s = w_beta[:, H:].sum(1)
    wb_raw = w_beta[:, :H] + np.outer(s, p_bar)
    wb_e = wb_raw * ln1_w[None, :]
    bb_e = b_beta + wb_raw @ ln1_b
    assert np.allclose(bb_e, 0.0), "nonzero folded beta bias not supported"

    wout_e = w_out * ln2_w[None, :]
    bout_e = b_out + w_out @ ln2_b
    assert np.allclose(bout_e, 0.0), "nonzero folded out bias not supported"

    w1_e = w1 * ln1_w[None, :]
    b1_e = b1 + w1 @ ln1_b
    assert np.allclose(b1_e, 0.0), "nonzero folded mlp1 bias not supported"

    # conv diag blocks: cd[p, ec, t, m] = conv_w[ec*128+p, 0, t] if p==m else 0
    cd = np.zeros((P, EC, 3, P), np.float32)
    idx = np.arange(P)
    cd[idx, :, :, idx] = conv_w[:, 0, :].reshape(EC, P, 3).transpose(1, 0, 2)
    # vector-conv weights for v: cwv[p, t, ec] = conv_w[ec*128+p, 0, t]
    cwv = np.ascontiguousarray(
        conv_w[:, 0, :].reshape(EC, P, 3).transpose(1, 2, 0)
    )

    w2T = _to_pchunk(w2.T, JC)  # [P, JC, H]

    shared = {
        "wq": bf(_col_chunks(_to_pchunk(wq_e.T, KC), 4)),
        "wk": bf(_col_chunks(_to_pchunk(wk_e.T, KC), 4)),
        "wv": bf(_col_chunks(_to_pchunk(wv_e.T, KC), 4)),
        "wb": bf(_col_chunks(_to_pchunk(wb_e.T, KC), 4)),
        "wo": bf(_col_chunks(_to_pchunk(wout_e.T, EC), 2)),
        "w1a": bf(_col_chunks(_to_pchunk(w1_e.T, KC)[:, :, :E], 4)),
        "w1b": bf(_col_chunks(_to_pchunk(w1_e.T, KC)[:, :, E:], 4)),
        "w2a": bf(np.ascontiguousarray(w2T[:, :, :NQ].reshape(P, 2, 16, NQ))),
        "w2b": bf(np.ascontiguousarray(w2T[:, :, NQ:].reshape(P, 2, 16, NQ))),
        "cdiag": bf(cd),
        "cwv": np.ascontiguousarray(cwv, dtype=np.float32),
    }
    in_maps = []
    for b in range(B):
        m = dict(shared)
        m["x"] = bf(x[b].reshape(LC, P, H).transpose(1, 0, 2))
        in_maps.append(m)
    return in_maps, attn_scale


def kernel(**inputs) -> np.ndarray:
    in_maps, attn_scale = _prep_inputs(inputs)
    nc = _build_program(attn_scale)
    _legalize_waits(nc)
    res = run_bass_kernel_spmd(nc, in_maps, core_ids=list(range(B)), trace=TRACE)
    LAST["exec_time_ns"] = res.exec_time_ns
    LAST["results"] = res
    out = np.empty((B, L, H), np.float32)
    for b in range(B):
        yb = np.asarray(res.results[b]["y"])  # [128, LC, H]
        out[b] = yb.transpose(1, 0, 2).reshape(L, H)
    return out
